# revision 15
# baseline (speedup 1.0000x reference)
"""Deformable conv (DCNv2) Trainium2 Bass kernel — v2.

Problem (hardcoded): x [8, 128, 64, 64] f32; offset/mask 3x3 convs (pad 1);
bilinear-gather im2col; GEMM with weights [256, 1152]; out [8, 256, 64, 64].

Sharding: data-parallel over batch N=8 across 8 NeuronCores (1 sample/core);
weights/conv params replicated.

Per-core pipeline (sample n):
  1. x -> SBUF; zero-padded bf16 conv input xpad [128, 68*68]; transposed
     padded image xt_pad [68*68(+70), 128] bf16 in DRAM (pad ring = 2 so
     out-of-range bilinear corners read zeros); 4 shifted DRAM->DRAM copies
     build xt4 [4625, 4, 128]: row r = the 4 bilinear corner pixel-rows
     {r, r+1, r+68, r+69} of an anchor, contiguous 1024B.
  2. PE: offset/mask conv as 9 shifted matmuls (27 out rows).
  3. PE-transpose conv output to j-major [j%128 part, (q, t)]; DVE coord
     math: floor, clamp, frac weights; mask*bilinear corner weights packed
     j-major as wpkb [128, k, t, r] bf16 (stay in SBUF — no broadcast);
     int16 anchor index per (tap, j) into xt4, wrapped for the gather.
  4. Per (jh, k) unit (18 total): NON-transpose SWDGE dma_gather (elem
     1024B) -> g [128 j-part, 16, 512] bf16, queues round-robin 0-3 with
     inline trigger (drains overlap freely - no xbar). DVE: one stride-0-
     broadcast mul by wpkb (scattered to corner planes) + 3 adds ->
     colT [j-part, t, c]. PE: 16 transposes (128x128, sub-bank PSUM) + 2
     ACT copies -> colC_k [c, (t, jp)] = im2col block in c-major, columns
     in sequential j order.
  5. PE GEMM accumulates over k; ACT bias epilogue; contiguous f32 store.
"""

import numpy as np
import ml_dtypes

import concourse.bass as bass
import concourse.mybir as mybir
import concourse.tile as tile
from concourse import bacc
from concourse.bass_utils import run_bass_kernel_spmd
from concourse.masks import make_identity
from concourse.tile_rust import add_dep_helper

F32 = mybir.dt.float32
BF16 = mybir.dt.bfloat16
I16 = mybir.dt.int16

N, C, H, W = 8, 128, 64, 64
K, K2, P = 3, 9, 256
HW = H * W                  # 4096
PW = W + 4                  # 68  (pad ring of 2)
ROWS = PW * PW              # 4624
ROWSP = ROWS + 70           # xt_pad rows (so xt4 row r can read r+69)
NT = HW // 128              # 32 j-tiles of 128
KT = K2 * NT                # 288
JH = HW // 2                # 2048 per j-half
NTH = NT // 2               # 16 t-chunks per j-half
MAGIC = 12582912.0          # 1.5 * 2**23: fp32 round-to-int magic

_CACHE = {}


def _build_nc():
    nc = bacc.Bacc("TRN2", target_bir_lowering=False, debug=False,
                   num_devices=N, num_swdge_queues=4)

    x_in = nc.dram_tensor("x", [C, HW], F32, kind="ExternalInput")
    lhsT_om = nc.dram_tensor("lhsT_om", [C, K2, 32], BF16, kind="ExternalInput")
    lhsT_gemm = nc.dram_tensor("lhsT_gemm", [C, K2, P], BF16, kind="ExternalInput")
    basey = nc.dram_tensor("basey", [128, KT], F32, kind="ExternalInput")
    basex = nc.dram_tensor("basex", [128, KT], F32, kind="ExternalInput")
    bias_col = nc.dram_tensor("bias_col", [128, 2], F32, kind="ExternalInput")
    y_out = nc.dram_tensor("y", [P, HW], F32, kind="ExternalOutput")

    with tile.TileContext(nc) as tc:
        with tc.tile_pool(name="dram", bufs=1, space="DRAM") as dram:
            xti = dram.tile([2 * (ROWS + 2) + 70, C], BF16)
            idxw_dram = dram.tile([16, K2 * 2 * 128], I16)
            _emit(tc, nc, x_in, lhsT_om, lhsT_gemm, basey, basex,
                  bias_col, y_out, xti, idxw_dram)
    nc.compile()
    return nc


def _emit(tc, nc, x_in, lhsT_om, lhsT_gemm, basey, basex, bias_col,
          y_out, xti, idxw_dram):
    TS = nc.vector.tensor_scalar
    TT_ADD = nc.vector.tensor_add
    TT_SUB = nc.vector.tensor_sub
    TT_MUL = nc.vector.tensor_mul
    Alu = mybir.AluOpType

    with tc.tile_pool(name="singles", bufs=1) as singles:
        # ---- persistent tiles ----
        om_sb = singles.tile([C, K2, 32], BF16, tag="om", name="om")
        gemm_sb = singles.tile([C, K2, P], BF16, tag="gemm_w", name="gemm_w")
        bias_sb = singles.tile([128, 2], F32, tag="bias", name="bias")
        idx_wr = singles.tile([128, K2, 2, 128], I16, tag="idx_wr", name="idx_wr")
        wpkb = singles.tile([128, K2, NT, 4], BF16, tag="wpkb", name="wpkb")
        wdup = singles.tile([128, K2, NT, 4, 2], BF16, tag="wdup", name="wdup")
        ident = singles.tile([128, 128], BF16, tag="ident", name="ident")
        identf = singles.tile([32, 32], F32, tag="identf", name="identf")
        identf128 = singles.tile([128, 128], F32, tag="identf128",
                                 name="identf128")

        nc.sync.dma_start(out=om_sb, in_=lhsT_om[:])
        nc.sync.dma_start(out=gemm_sb, in_=lhsT_gemm[:])
        nc.sync.dma_start(out=bias_sb, in_=bias_col[:])
        make_identity(nc, ident)
        make_identity(nc, identf)
        make_identity(nc, identf128)

        with tc.tile_pool(name="stage1", bufs=1) as st1, \
             tc.tile_pool(name="coord", bufs=1) as coord, \
             tc.tile_pool(name="ps_a", bufs=2, space="PSUM") as ps_a, \
             tc.tile_pool(name="trbuf", bufs=4) as trbuf:

            # ---- stage 1: load x, build xpad (SBUF) and xt_pad (DRAM) ----
            xpad = st1.tile([C, ROWS], BF16, tag="xpad", name="xpad")
            x_sb = st1.tile([C, HW], F32, tag="x", name="x")
            nc.sync.dma_start(out=x_sb, in_=x_in[:])

            nc.vector.memset(xpad, 0.0)
            xpad_int = bass.AP(tensor=xpad.tensor,
                               offset=xpad.offset + 2 * PW + 2,
                               ap=[xpad.ap[0], [PW, H], [1, W]])
            nc.scalar.copy(out=xpad_int,
                           in_=x_sb[:].rearrange("c (h w) -> c h w", h=H))

            xbf = st1.tile([C, HW], BF16, tag="xbf", name="xbf")
            nc.vector.tensor_copy(xbf, x_sb)

            # zero xtI (2*(ROWS+2)+70 rows x C) via overlapping flat DMAs
            NZI = (2 * (ROWS + 2) + 70) * C
            zt = st1.tile([128, 4700], BF16, tag="zt", name="zt")
            nc.vector.memset(zt, 0.0)
            zsrc = bass.AP(tensor=zt.tensor, offset=zt.offset,
                           ap=[[zt.ap[0][0], 128], [1, 4700]])
            half = 128 * 4700
            nc.sync.dma_start(
                out=bass.AP(tensor=xti.tensor, offset=0, ap=[[1, half]]),
                in_=zsrc)
            nc.scalar.dma_start(
                out=bass.AP(tensor=xti.tensor,
                            offset=NZI - half, ap=[[1, half]]),
                in_=zsrc)

            # transpose x (bf16) 128-col chunks -> xtI interior, 2x row-
            # interleaved: xtI[2i] = padded row i, xtI[2i+1] = padded row
            # i+68, so the 4 bilinear corners of anchor r are the 4
            # consecutive rows xtI[2r..2r+3] (one contiguous 1024B gather
            # element at stride 512B). Each pixel row is written twice
            # (even slot of its own anchor, odd slot of the anchor 68
            # above). Writes alternate the SP/ACT HWDGE rings.
            for t in range(NT):
                tr_ps = ps_a.tile([128, 128], BF16, tag="trx", name="trx")
                nc.tensor.transpose(tr_ps[:], xbf[:, t * 128:(t + 1) * 128],
                                    ident[:])
                tr_sb = trbuf.tile([128, 128], BF16, tag="trx_sb", name="trx_sb")
                nc.scalar.copy(out=tr_sb, in_=tr_ps)
                src = bass.AP(tensor=tr_sb.tensor, offset=tr_sb.offset,
                              ap=[[tr_sb.ap[0][0], 128], [1, 128]])
                q0 = (2 * t + 2) * PW + 2
                for half_i, roff in enumerate((2 * q0, 2 * (q0 - PW) + 1)):
                    dst = bass.AP(tensor=xti.tensor,
                                  offset=roff * C,
                                  ap=[[PW * 2 * C, 2], [2 * C, W], [1, C]])
                    eng = nc.sync if (t + half_i) % 2 == 0 else nc.scalar
                    eng.dma_start(out=dst, in_=src)

            # ---- stage 2: offset/mask conv (27 out rows), 512-col chunks ----
            co_sb = st1.tile([32, HW], F32, tag="co", name="co")
            for nt8 in range(8):
                co_ps = ps_a.tile([32, 512], F32, tag="conv", name="conv")
                for tap in range(K2):
                    dy, dx = tap // K, tap % K
                    rhs = bass.AP(
                        tensor=xpad.tensor,
                        offset=(xpad.offset + (1 + dy) * PW + (1 + dx)
                                + (nt8 * 8) * PW),
                        ap=[xpad.ap[0], [PW, 8], [1, W]],
                    )
                    nc.tensor.matmul(co_ps[:], om_sb[:, tap, :], rhs,
                                     start=(tap == 0), stop=(tap == K2 - 1))
                nc.scalar.copy(out=co_sb[:, nt8 * 512:(nt8 + 1) * 512],
                               in_=co_ps)

            # ---- stage 3: transpose conv out to j-major; coordinate math ----
            trj = coord.tile([128, 32, NT], F32, tag="trj", name="trj")   # [jp, q, t]
            for t in range(NT):
                tp = ps_a.tile([128, 32], F32, tag="trjp", name="trjp")
                nc.tensor.transpose(tp[:], co_sb[:, t * 128:(t + 1) * 128],
                                    identf[:])
                nc.vector.tensor_copy(trj[:, :, t], tp)

            dy_all = trj[:, 0:K2, :]
            dx_all = trj[:, K2:2 * K2, :]
            m_all = trj[:, 2 * K2:3 * K2, :]

            by = coord.tile([128, KT], F32, tag="by", name="by")
            bx = coord.tile([128, KT], F32, tag="bx", name="bx")
            nc.sync.dma_start(out=by, in_=basey[:])
            nc.sync.dma_start(out=bx, in_=basex[:])

            def f32t(tag):
                return coord.tile([128, KT], F32, tag=tag, name=tag)

            py = f32t("py"); TT_ADD(py, dy_all, by)
            px = f32t("px"); TT_ADD(px, dx_all, bx)
            ty = f32t("ty"); TS(out=ty, in0=py, scalar1=-0.5, scalar2=MAGIC,
                                op0=Alu.add, op1=Alu.add)
            y0 = f32t("y0"); TS(out=y0, in0=ty, scalar1=MAGIC, scalar2=None,
                                op0=Alu.subtract)
            tx = f32t("tx"); TS(out=tx, in0=px, scalar1=-0.5, scalar2=MAGIC,
                                op0=Alu.add, op1=Alu.add)
            x0 = f32t("x0"); TS(out=x0, in0=tx, scalar1=MAGIC, scalar2=None,
                                op0=Alu.subtract)
            ly = f32t("ly"); TT_SUB(ly, py, y0)
            lx = f32t("lx"); TT_SUB(lx, px, x0)
            y0c = f32t("y0c"); TS(out=y0c, in0=y0, scalar1=-2.0, scalar2=64.0,
                                  op0=Alu.max, op1=Alu.min)
            x0c = f32t("x0c"); TS(out=x0c, in0=x0, scalar1=-2.0, scalar2=64.0,
                                  op0=Alu.max, op1=Alu.min)

            # idx = (y0c+2)*68 + (x0c+2)
            ia = f32t("ia"); TS(out=ia, in0=y0c, scalar1=float(PW),
                                scalar2=float(2 * PW + 2),
                                op0=Alu.mult, op1=Alu.add)
            idx0f = f32t("idx0f"); TT_ADD(idx0f, ia, x0c)

            # mask * bilinear corner weights (mask = 2*sigmoid(conv)),
            # packed j-major into wpack[jp, k, t, r]; corner order matches
            # xt4: r0=(y0,x0) r1=(y0,x0+1) r2=(y0+1,x0) r3=(y0+1,x0+1)
            sig = f32t("sig")
            nc.scalar.activation(out=sig, in_=m_all,
                                 func=mybir.ActivationFunctionType.Sigmoid)
            m2 = f32t("m2"); TS(out=m2, in0=sig, scalar1=2.0, scalar2=None,
                                op0=Alu.mult)
            mly = f32t("mly"); TT_MUL(mly, m2, ly)
            muy = f32t("muy"); TT_SUB(muy, m2, mly)

            wpack = coord.tile([128, K2, NT, 4], F32, tag="wpack", name="wpack")

            def wslice(r):
                return bass.AP(tensor=wpack.tensor,
                               offset=wpack.offset + r,
                               ap=[wpack.ap[0], [NT * 4, K2], [4, NT]])

            def v3(t):  # [128, KT] -> [128, K2, NT] view
                return t[:].rearrange("p (k t) -> p k t", k=K2)

            TT_MUL(wslice(3), v3(mly), v3(lx))
            TT_SUB(wslice(1), v3(mly), wslice(3))
            TT_MUL(wslice(2), v3(muy), v3(lx))
            TT_SUB(wslice(0), v3(muy), wslice(2))
            nc.vector.tensor_copy(wpkb, wpack)
            wde = bass.AP(tensor=wdup.tensor, offset=wdup.offset,
                          ap=[wdup.ap[0], [2, K2 * NT * 4], [1, 2]])
            wps = bass.AP(tensor=wpkb.tensor, offset=wpkb.offset,
                          ap=[wpkb.ap[0], [1, K2 * NT * 4], [0, 2]])
            nc.vector.tensor_copy(wde, wps)

            # wrap indices for the non-transpose gather: descriptor number
            # i = t*128 + p (j = jh*2048 + i) read from wrapped [i%16, i//16]
            # = [p%16, t*8 + p//16] with p = 16a + b. The 8-way a-interleave
            # crosses partition groups, so route through PE transposes:
            # idx0f k-chunks [128, 32] -> [32 t, 128 jp] PSUM, ACT-scatter
            # the jp axis to (b*8 + a) order, cast i16, then 18 clean 3-D
            # SBUF->DRAM writes into wrapped layout + 1 broadcast back.
            idxts = coord.tile([32, K2, 128], F32, tag="idxts", name="idxts")
            for k in range(K2):
                tp2 = ps_a.tile([32, 128], F32, tag="idxtp", name="idxtp")
                nc.tensor.transpose(tp2[:], idx0f[:, k * NT:(k + 1) * NT],
                                    identf128[:])
                # scatter jp = 16a+b -> free pos b*8 + a
                dsc = bass.AP(tensor=idxts.tensor,
                              offset=idxts.offset + k * 128,
                              ap=[idxts.ap[0], [1, 8], [8, 16]])
                nc.scalar.copy(out=dsc, in_=tp2[:].rearrange("t (a b) -> t a b",
                                                             a=8))
            idxts_i = coord.tile([32, K2, 128], I16, tag="idxts_i",
                                 name="idxts_i")
            nc.vector.tensor_copy(idxts_i, idxts)
            for k in range(K2):
                for jh in range(2):
                    sbt = idxts_i[jh * NTH:(jh + 1) * NTH, :, :]
                    src = bass.AP(tensor=sbt.tensor,
                                  offset=sbt.offset + k * 128,
                                  ap=[sbt.ap[0], [8, 16], [1, 8]])
                    dst = bass.AP(tensor=idxw_dram.tensor,
                                  offset=(idxw_dram.offset + k * 256
                                          + jh * 128),
                                  ap=[[8, NTH], [K2 * 256, 16], [1, 8]])
                    nc.sync.dma_start(out=dst, in_=src)
            FW = K2 * 2 * 128
            bsrc = bass.AP(tensor=idxw_dram.tensor, offset=idxw_dram.offset,
                           ap=[[0, 8], [FW, 16], [1, FW]])
            idst = bass.AP(tensor=idx_wr.tensor, offset=idx_wr.offset,
                           ap=[[idx_wr.ap[0][0], 128], [1, FW]])
            nc.sync.dma_start(out=idst, in_=bsrc)

        # ---- stages 4+5: per (jh, k): gather -> combine -> transpose; GEMM
        gsrc = bass.AP(tensor=xti.tensor, offset=xti.offset,
                       ap=[[2 * C, ROWS + 2], [1, 4 * C]])
        sems = [nc.alloc_semaphore(f"swdge_q{q}") for q in range(4)]
        drains = [0, 0, 0, 0]

        with tc.tile_pool(name="gw", bufs=4) as gw, \
             tc.tile_pool(name="g2p", bufs=2) as g2p, \
             tc.tile_pool(name="ctp", bufs=2) as ctp, \
             tc.tile_pool(name="colp", bufs=1) as colp, \
             tc.tile_pool(name="outp", bufs=2) as outp, \
             tc.tile_pool(name="ps_t", bufs=2, space="PSUM") as ps_t, \
             tc.tile_pool(name="ps_b", bufs=1, space="PSUM") as ps_b:

            cols = []
            for u, (jh, k) in enumerate(
                    [(jh, k) for jh in range(2) for k in range(K2)]):
                q = u % 4
                g = gw.tile([128, NTH, 4 * C], BF16, tag="g", name="g")
                gi = nc.gpsimd.dma_gather(
                    out_ap=g[:],
                    in_ap=gsrc,
                    idxs_ap=idx_wr[:, k, jh, :],
                    num_idxs=JH,
                    num_idxs_reg=JH,
                    elem_size=4 * C,
                    elem_step=2 * C,
                    transpose=False,
                    single_packet=False,
                    queue_num=q,
                )
                gi.then_inc(sems[q], 16)
                drains[q] += 1
                v_w = nc.vector.wait_ge(sems[q], 16 * drains[q])

                # combine: 4 per-corner contiguous muls (per-partition
                # weights broadcast along c), then 3 adds -> colT [jp, t, c]
                g2 = g2p.tile([128, 4, NTH, C], BF16, tag="g2", name="g2")
                for r in range(4):
                    g_r = bass.AP(tensor=g.tensor, offset=g.offset + r * C,
                                  ap=[g.ap[0], [4 * C, NTH], [2, C // 2],
                                      [1, 2]])
                    wb_r = bass.AP(tensor=wdup.tensor,
                                   offset=(wdup.offset
                                           + ((k * NT + jh * NTH) * 4 + r) * 2),
                                   ap=[wdup.ap[0], [8, NTH], [0, C // 2],
                                       [1, 2]])
                    o_r = bass.AP(tensor=g2.tensor,
                                  offset=g2.offset + r * NTH * C,
                                  ap=[g2.ap[0], [C, NTH], [2, C // 2], [1, 2]])
                    mul = TT_MUL(o_r, g_r, wb_r)
                    if r == 0:
                        add_dep_helper(mul.ins, v_w.ins, sync=False)
                colT = ctp.tile([128, NTH, C], BF16, tag="colT", name="colT")
                TT_ADD(colT, g2[:, 0], g2[:, 1])
                TT_ADD(colT, colT, g2[:, 2])
                TT_ADD(colT, colT, g2[:, 3])

                # PE-transpose to c-major: colC[c, t, jp], j = jh*2048+t*128+jp
                colC = colp.tile([128, NTH, 128], BF16, tag=f"colC{k}",
                                 name=f"colC{k}")
                for h in range(2):
                    tp = ps_t.tile([128, 8, 128], BF16, tag="tp", name="tp")
                    for s in range(8):
                        nc.tensor.transpose(tp[:, s, :], colT[:, 8 * h + s, :],
                                            ident[:])
                    nc.scalar.copy(out=colC[:, 8 * h:8 * (h + 1), :], in_=tp)
                cols.append(colC)
                if k != K2 - 1:
                    continue

                # ---- GEMM for this j-half ----
                for m in range(2):
                    ps_n = [ps_b.tile([128, 512], F32, tag=f"gemm{n2}",
                                      name=f"gemm{n2}") for n2 in range(4)]
                    for kk in range(K2):
                        ck = cols[kk][:].rearrange("p t c -> p (t c)")
                        for n2 in range(4):
                            nc.tensor.matmul(
                                ps_n[n2][:],
                                gemm_sb[:, kk, m * 128:(m + 1) * 128],
                                ck[:, n2 * 512:(n2 + 1) * 512],
                                start=(kk == 0), stop=(kk == K2 - 1),
                            )
                    o_sb = outp.tile([128, JH], F32, tag="o", name="o")
                    for n2 in range(4):
                        nc.scalar.activation(
                            out=o_sb[:, n2 * 512:(n2 + 1) * 512], in_=ps_n[n2],
                            func=mybir.ActivationFunctionType.Identity,
                            bias=bias_sb[:, m:m + 1])
                    dst = bass.AP(tensor=y_out,
                                  offset=m * 128 * HW + jh * JH,
                                  ap=[[HW, 128], [1, JH]])
                    nc.sync.dma_start(out=dst, in_=o_sb)
                cols = []


def _host_constants():
    if "consts" in _CACHE:
        return _CACHE["consts"]
    t_idx = np.arange(NT)
    p_idx = np.arange(128)
    j = t_idx[None, :] * 128 + p_idx[:, None]          # [128, 32]
    iy = j // W
    ix = j % W
    ky = np.repeat(np.arange(K), K)
    kx = np.tile(np.arange(K), K)
    basey = np.zeros((128, KT), dtype=np.float32)
    basex = np.zeros((128, KT), dtype=np.float32)
    for k in range(K2):
        basey[:, k * NT:(k + 1) * NT] = iy - 1 + ky[k]
        basex[:, k * NT:(k + 1) * NT] = ix - 1 + kx[k]
    _CACHE["consts"] = (basey, basex)
    return _CACHE["consts"]


def kernel(x, offset_w, offset_b, mask_w, mask_b, weights, bias):
    x = np.asarray(x, dtype=np.float32)
    offset_w = np.asarray(offset_w, dtype=np.float32)
    mask_w = np.asarray(mask_w, dtype=np.float32)
    weights = np.asarray(weights, dtype=np.float32)
    bias = np.asarray(bias, dtype=np.float32)
    offset_b = np.asarray(offset_b, dtype=np.float32)
    mask_b = np.asarray(mask_b, dtype=np.float32)
    assert np.all(offset_b == 0) and np.all(mask_b == 0), "zero conv bias assumed"

    if "nc" not in _CACHE:
        _CACHE["nc"] = _build_nc()
    nc = _CACHE["nc"]
    basey, basex = _host_constants()

    # offset/mask conv stationary operand [c, tap, q]: q 0-8 dy, 9-17 dx, 18-26 m
    lhsT_om = np.zeros((C, K2, 32), dtype=np.float32)
    ow = offset_w.reshape(K2, 2, C, K, K)
    for tap in range(K2):
        dy, dx = tap // K, tap % K
        lhsT_om[:, tap, 0:K2] = ow[:, 0, :, dy, dx].T
        lhsT_om[:, tap, K2:2 * K2] = ow[:, 1, :, dy, dx].T
        lhsT_om[:, tap, 2 * K2:3 * K2] = mask_w[:, :, dy, dx].T
    lhsT_om = lhsT_om.astype(ml_dtypes.bfloat16)

    # GEMM stationary operand: lhsT_gemm[k, c, p] = weights[p, c*9 + k]
    wr = weights.reshape(P, C, K2)
    lhsT_gemm = np.ascontiguousarray(wr.transpose(1, 2, 0)).astype(ml_dtypes.bfloat16)

    bias_col = np.ascontiguousarray(bias.reshape(2, 128).T).astype(np.float32)

    in_maps = []
    for n in range(N):
        in_maps.append({
            "x": np.ascontiguousarray(x[n].reshape(C, HW)),
            "lhsT_om": lhsT_om,
            "lhsT_gemm": lhsT_gemm,
            "basey": basey,
            "basex": basex,
            "bias_col": bias_col,
        })

    res = run_bass_kernel_spmd(nc, in_maps, core_ids=list(range(N)),
                               trace=bool(_CACHE.get("trace")),
                               trace_cores=_CACHE.get("trace_cores"))
    _CACHE["last_res"] = res
    out = np.stack([res.results[n]["y"].reshape(P, H, W) for n in range(N)])
    return out.astype(np.float32)


# revision 16
# speedup vs baseline: 1.0480x; 1.0480x over previous
"""Deformable conv (DCNv2) Trainium2 Bass kernel — v2.

Problem (hardcoded): x [8, 128, 64, 64] f32; offset/mask 3x3 convs (pad 1);
bilinear-gather im2col; GEMM with weights [256, 1152]; out [8, 256, 64, 64].

Sharding: data-parallel over batch N=8 across 8 NeuronCores (1 sample/core);
weights/conv params replicated.

Per-core pipeline (sample n):
  1. x -> SBUF; zero-padded bf16 conv input xpad [128, 68*68]; transposed
     padded image xt_pad [68*68(+70), 128] bf16 in DRAM (pad ring = 2 so
     out-of-range bilinear corners read zeros); 4 shifted DRAM->DRAM copies
     build xt4 [4625, 4, 128]: row r = the 4 bilinear corner pixel-rows
     {r, r+1, r+68, r+69} of an anchor, contiguous 1024B.
  2. PE: offset/mask conv as 9 shifted matmuls (27 out rows).
  3. PE-transpose conv output to j-major [j%128 part, (q, t)]; DVE coord
     math: floor, clamp, frac weights; mask*bilinear corner weights packed
     j-major as wpkb [128, k, t, r] bf16 (stay in SBUF — no broadcast);
     int16 anchor index per (tap, j) into xt4, wrapped for the gather.
  4. Per (jh, k) unit (18 total): NON-transpose SWDGE dma_gather (elem
     1024B) -> g [128 j-part, 16, 512] bf16, queues round-robin 0-3 with
     inline trigger (drains overlap freely - no xbar). DVE: one stride-0-
     broadcast mul by wpkb (scattered to corner planes) + 3 adds ->
     colT [j-part, t, c]. PE: 16 transposes (128x128, sub-bank PSUM) + 2
     ACT copies -> colC_k [c, (t, jp)] = im2col block in c-major, columns
     in sequential j order.
  5. PE GEMM accumulates over k; ACT bias epilogue; contiguous f32 store.
"""

import numpy as np
import ml_dtypes

import concourse.bass as bass
import concourse.mybir as mybir
import concourse.tile as tile
from concourse import bacc
from concourse.bass_utils import run_bass_kernel_spmd
from concourse.masks import make_identity
from concourse.tile_rust import add_dep_helper

F32 = mybir.dt.float32
BF16 = mybir.dt.bfloat16
I16 = mybir.dt.int16

N, C, H, W = 8, 128, 64, 64
K, K2, P = 3, 9, 256
HW = H * W                  # 4096
PW = W + 4                  # 68  (pad ring of 2)
ROWS = PW * PW              # 4624
ROWSP = ROWS + 70           # xt_pad rows (so xt4 row r can read r+69)
NT = HW // 128              # 32 j-tiles of 128
KT = K2 * NT                # 288
JH = HW // 2                # 2048 per j-half
NTH = NT // 2               # 16 t-chunks per j-half
MAGIC = 12582912.0          # 1.5 * 2**23: fp32 round-to-int magic

_CACHE = {}


def _build_nc():
    nc = bacc.Bacc("TRN2", target_bir_lowering=False, debug=False,
                   num_devices=N, num_swdge_queues=4)

    x_in = nc.dram_tensor("x", [C, HW], F32, kind="ExternalInput")
    lhsT_om = nc.dram_tensor("lhsT_om", [C, K2, 32], BF16, kind="ExternalInput")
    lhsT_gemm = nc.dram_tensor("lhsT_gemm", [C, K2, P], BF16, kind="ExternalInput")
    basey = nc.dram_tensor("basey", [128, KT], F32, kind="ExternalInput")
    basex = nc.dram_tensor("basex", [128, KT], F32, kind="ExternalInput")
    bias_col = nc.dram_tensor("bias_col", [128, 2], F32, kind="ExternalInput")
    y_out = nc.dram_tensor("y", [P, HW], F32, kind="ExternalOutput")

    with tile.TileContext(nc) as tc:
        with tc.tile_pool(name="dram", bufs=1, space="DRAM") as dram:
            xti = dram.tile([2 * (ROWS + 2) + 70, C], BF16)
            idxw_dram = dram.tile([16, K2 * 2 * 128], I16)
            _emit(tc, nc, x_in, lhsT_om, lhsT_gemm, basey, basex,
                  bias_col, y_out, xti, idxw_dram)
    nc.compile()
    return nc


def _emit(tc, nc, x_in, lhsT_om, lhsT_gemm, basey, basex, bias_col,
          y_out, xti, idxw_dram):
    TS = nc.vector.tensor_scalar
    TT_ADD = nc.vector.tensor_add
    TT_SUB = nc.vector.tensor_sub
    TT_MUL = nc.vector.tensor_mul
    Alu = mybir.AluOpType

    with tc.tile_pool(name="singles", bufs=1) as singles:
        # ---- persistent tiles ----
        om_sb = singles.tile([C, K2, 32], BF16, tag="om", name="om")
        gemm_sb = singles.tile([C, K2, P], BF16, tag="gemm_w", name="gemm_w")
        bias_sb = singles.tile([128, 2], F32, tag="bias", name="bias")
        idx_wr = singles.tile([128, K2, 2, 128], I16, tag="idx_wr", name="idx_wr")
        wpkb = singles.tile([128, K2, NT, 4], BF16, tag="wpkb", name="wpkb")
        wdup = singles.tile([128, K2, NT, 4, 2], BF16, tag="wdup", name="wdup")
        ident = singles.tile([128, 128], BF16, tag="ident", name="ident")
        identf = singles.tile([32, 32], F32, tag="identf", name="identf")
        identf128 = singles.tile([128, 128], F32, tag="identf128",
                                 name="identf128")

        nc.sync.dma_start(out=om_sb, in_=lhsT_om[:])
        nc.sync.dma_start(out=gemm_sb, in_=lhsT_gemm[:])
        nc.sync.dma_start(out=bias_sb, in_=bias_col[:])
        make_identity(nc, ident)
        make_identity(nc, identf)
        make_identity(nc, identf128)

        with tc.tile_pool(name="stage1", bufs=1) as st1, \
             tc.tile_pool(name="coord", bufs=1) as coord, \
             tc.tile_pool(name="ps_a", bufs=2, space="PSUM") as ps_a, \
             tc.tile_pool(name="trbuf", bufs=4) as trbuf:

            # ---- stage 1: load x, build xpad (SBUF) and xt_pad (DRAM) ----
            xpad = st1.tile([C, ROWS], BF16, tag="xpad", name="xpad")
            x_sb = st1.tile([C, HW], F32, tag="x", name="x")
            nc.sync.dma_start(out=x_sb, in_=x_in[:])

            nc.vector.memset(xpad, 0.0)
            xpad_int = bass.AP(tensor=xpad.tensor,
                               offset=xpad.offset + 2 * PW + 2,
                               ap=[xpad.ap[0], [PW, H], [1, W]])
            nc.scalar.copy(out=xpad_int,
                           in_=x_sb[:].rearrange("c (h w) -> c h w", h=H))

            xbf = st1.tile([C, HW], BF16, tag="xbf", name="xbf")
            nc.vector.tensor_copy(xbf, x_sb)

            # zero xtI (2*(ROWS+2)+70 rows x C) via overlapping flat DMAs
            NZI = (2 * (ROWS + 2) + 70) * C
            zt = st1.tile([128, 4700], BF16, tag="zt", name="zt")
            nc.vector.memset(zt, 0.0)
            zsrc = bass.AP(tensor=zt.tensor, offset=zt.offset,
                           ap=[[zt.ap[0][0], 128], [1, 4700]])
            half = 128 * 4700
            nc.sync.dma_start(
                out=bass.AP(tensor=xti.tensor, offset=0, ap=[[1, half]]),
                in_=zsrc)
            nc.scalar.dma_start(
                out=bass.AP(tensor=xti.tensor,
                            offset=NZI - half, ap=[[1, half]]),
                in_=zsrc)

            # transpose x (bf16) 128-col chunks -> xtI interior, 2x row-
            # interleaved: xtI[2i] = padded row i, xtI[2i+1] = padded row
            # i+68, so the 4 bilinear corners of anchor r are the 4
            # consecutive rows xtI[2r..2r+3] (one contiguous 1024B gather
            # element at stride 512B). Each pixel row is written twice
            # (even slot of its own anchor, odd slot of the anchor 68
            # above). Writes alternate the SP/ACT HWDGE rings.
            for t in range(NT):
                tr_ps = ps_a.tile([128, 128], BF16, tag="trx", name="trx")
                nc.tensor.transpose(tr_ps[:], xbf[:, t * 128:(t + 1) * 128],
                                    ident[:])
                tr_sb = trbuf.tile([128, 128], BF16, tag="trx_sb", name="trx_sb")
                nc.scalar.copy(out=tr_sb, in_=tr_ps)
                src = bass.AP(tensor=tr_sb.tensor, offset=tr_sb.offset,
                              ap=[[tr_sb.ap[0][0], 128], [1, 128]])
                q0 = (2 * t + 2) * PW + 2
                for half_i, roff in enumerate((2 * q0, 2 * (q0 - PW) + 1)):
                    dst = bass.AP(tensor=xti.tensor,
                                  offset=roff * C,
                                  ap=[[PW * 2 * C, 2], [2 * C, W], [1, C]])
                    nc.sync.dma_start(out=dst, in_=src)

            # ---- stage 2: offset/mask conv (27 out rows), 512-col chunks ----
            co_sb = st1.tile([32, HW], F32, tag="co", name="co")
            for nt8 in range(8):
                co_ps = ps_a.tile([32, 512], F32, tag="conv", name="conv")
                for tap in range(K2):
                    dy, dx = tap // K, tap % K
                    rhs = bass.AP(
                        tensor=xpad.tensor,
                        offset=(xpad.offset + (1 + dy) * PW + (1 + dx)
                                + (nt8 * 8) * PW),
                        ap=[xpad.ap[0], [PW, 8], [1, W]],
                    )
                    nc.tensor.matmul(co_ps[:], om_sb[:, tap, :], rhs,
                                     start=(tap == 0), stop=(tap == K2 - 1))
                nc.scalar.copy(out=co_sb[:, nt8 * 512:(nt8 + 1) * 512],
                               in_=co_ps)

            # ---- stage 3: transpose conv out to j-major; coordinate math ----
            trj = coord.tile([128, 32, NT], F32, tag="trj", name="trj")   # [jp, q, t]
            for t in range(NT):
                tp = ps_a.tile([128, 32], F32, tag="trjp", name="trjp")
                nc.tensor.transpose(tp[:], co_sb[:, t * 128:(t + 1) * 128],
                                    identf[:])
                nc.vector.tensor_copy(trj[:, :, t], tp)

            dy_all = trj[:, 0:K2, :]
            dx_all = trj[:, K2:2 * K2, :]
            m_all = trj[:, 2 * K2:3 * K2, :]

            by = coord.tile([128, KT], F32, tag="by", name="by")
            bx = coord.tile([128, KT], F32, tag="bx", name="bx")
            nc.sync.dma_start(out=by, in_=basey[:])
            nc.sync.dma_start(out=bx, in_=basex[:])

            def f32t(tag):
                return coord.tile([128, KT], F32, tag=tag, name=tag)

            py = f32t("py"); TT_ADD(py, dy_all, by)
            px = f32t("px"); TT_ADD(px, dx_all, bx)
            ty = f32t("ty"); TS(out=ty, in0=py, scalar1=-0.5, scalar2=MAGIC,
                                op0=Alu.add, op1=Alu.add)
            y0 = f32t("y0"); TS(out=y0, in0=ty, scalar1=MAGIC, scalar2=None,
                                op0=Alu.subtract)
            tx = f32t("tx"); TS(out=tx, in0=px, scalar1=-0.5, scalar2=MAGIC,
                                op0=Alu.add, op1=Alu.add)
            x0 = f32t("x0"); TS(out=x0, in0=tx, scalar1=MAGIC, scalar2=None,
                                op0=Alu.subtract)
            ly = f32t("ly"); TT_SUB(ly, py, y0)
            lx = f32t("lx"); TT_SUB(lx, px, x0)
            y0c = f32t("y0c"); TS(out=y0c, in0=y0, scalar1=-2.0, scalar2=64.0,
                                  op0=Alu.max, op1=Alu.min)
            x0c = f32t("x0c"); TS(out=x0c, in0=x0, scalar1=-2.0, scalar2=64.0,
                                  op0=Alu.max, op1=Alu.min)

            # idx = (y0c+2)*68 + (x0c+2)
            ia = f32t("ia"); TS(out=ia, in0=y0c, scalar1=float(PW),
                                scalar2=float(2 * PW + 2),
                                op0=Alu.mult, op1=Alu.add)
            idx0f = f32t("idx0f"); TT_ADD(idx0f, ia, x0c)

            # mask * bilinear corner weights (mask = 2*sigmoid(conv)),
            # packed j-major into wpack[jp, k, t, r]; corner order matches
            # xt4: r0=(y0,x0) r1=(y0,x0+1) r2=(y0+1,x0) r3=(y0+1,x0+1)
            sig = f32t("sig")
            nc.scalar.activation(out=sig, in_=m_all,
                                 func=mybir.ActivationFunctionType.Sigmoid)
            m2 = f32t("m2"); TS(out=m2, in0=sig, scalar1=2.0, scalar2=None,
                                op0=Alu.mult)
            mly = f32t("mly"); TT_MUL(mly, m2, ly)
            muy = f32t("muy"); TT_SUB(muy, m2, mly)

            wpack = coord.tile([128, K2, NT, 4], F32, tag="wpack", name="wpack")

            def wslice(r):
                return bass.AP(tensor=wpack.tensor,
                               offset=wpack.offset + r,
                               ap=[wpack.ap[0], [NT * 4, K2], [4, NT]])

            def v3(t):  # [128, KT] -> [128, K2, NT] view
                return t[:].rearrange("p (k t) -> p k t", k=K2)

            TT_MUL(wslice(3), v3(mly), v3(lx))
            TT_SUB(wslice(1), v3(mly), wslice(3))
            TT_MUL(wslice(2), v3(muy), v3(lx))
            TT_SUB(wslice(0), v3(muy), wslice(2))
            nc.vector.tensor_copy(wpkb, wpack)
            wde = bass.AP(tensor=wdup.tensor, offset=wdup.offset,
                          ap=[wdup.ap[0], [2, K2 * NT * 4], [1, 2]])
            wps = bass.AP(tensor=wpkb.tensor, offset=wpkb.offset,
                          ap=[wpkb.ap[0], [1, K2 * NT * 4], [0, 2]])
            nc.vector.tensor_copy(wde, wps)

            # wrap indices for the non-transpose gather: descriptor number
            # i = t*128 + p (j = jh*2048 + i) read from wrapped [i%16, i//16]
            # = [p%16, t*8 + p//16] with p = 16a + b. The 8-way a-interleave
            # crosses partition groups, so route through PE transposes:
            # idx0f k-chunks [128, 32] -> [32 t, 128 jp] PSUM, ACT-scatter
            # the jp axis to (b*8 + a) order, cast i16, then 18 clean 3-D
            # SBUF->DRAM writes into wrapped layout + 1 broadcast back.
            idxts = coord.tile([32, K2, 128], F32, tag="idxts", name="idxts")
            for k in range(K2):
                tp2 = ps_a.tile([32, 128], F32, tag="idxtp", name="idxtp")
                nc.tensor.transpose(tp2[:], idx0f[:, k * NT:(k + 1) * NT],
                                    identf128[:])
                # scatter jp = 16a+b -> free pos b*8 + a
                dsc = bass.AP(tensor=idxts.tensor,
                              offset=idxts.offset + k * 128,
                              ap=[idxts.ap[0], [1, 8], [8, 16]])
                nc.scalar.copy(out=dsc, in_=tp2[:].rearrange("t (a b) -> t a b",
                                                             a=8))
            idxts_i = coord.tile([32, K2, 128], I16, tag="idxts_i",
                                 name="idxts_i")
            nc.vector.tensor_copy(idxts_i, idxts)
            for k in range(K2):
                for jh in range(2):
                    sbt = idxts_i[jh * NTH:(jh + 1) * NTH, :, :]
                    src = bass.AP(tensor=sbt.tensor,
                                  offset=sbt.offset + k * 128,
                                  ap=[sbt.ap[0], [8, 16], [1, 8]])
                    dst = bass.AP(tensor=idxw_dram.tensor,
                                  offset=(idxw_dram.offset + k * 256
                                          + jh * 128),
                                  ap=[[8, NTH], [K2 * 256, 16], [1, 8]])
                    nc.sync.dma_start(out=dst, in_=src)
            FW = K2 * 2 * 128
            bsrc = bass.AP(tensor=idxw_dram.tensor, offset=idxw_dram.offset,
                           ap=[[0, 8], [FW, 16], [1, FW]])
            idst = bass.AP(tensor=idx_wr.tensor, offset=idx_wr.offset,
                           ap=[[idx_wr.ap[0][0], 128], [1, FW]])
            nc.sync.dma_start(out=idst, in_=bsrc)

        # ---- stages 4+5: per (jh, k): gather -> combine -> transpose; GEMM
        gsrc = bass.AP(tensor=xti.tensor, offset=xti.offset,
                       ap=[[2 * C, ROWS + 2], [1, 4 * C]])
        sems = [nc.alloc_semaphore(f"swdge_q{q}") for q in range(4)]
        drains = [0, 0, 0, 0]

        with tc.tile_pool(name="gw", bufs=4) as gw, \
             tc.tile_pool(name="g2p", bufs=1) as g2p, \
             tc.tile_pool(name="ctp", bufs=2) as ctp, \
             tc.tile_pool(name="colp", bufs=2) as colp, \
             tc.tile_pool(name="outp", bufs=2) as outp, \
             tc.tile_pool(name="ps_t", bufs=2, space="PSUM") as ps_t, \
             tc.tile_pool(name="ps_b", bufs=1, space="PSUM") as ps_b:

            cols = []
            for u, (jh, k) in enumerate(
                    [(jh, k) for jh in range(2) for k in range(K2)]):
                q = u % 4
                g = gw.tile([128, NTH, 4 * C], BF16, tag="g", name="g")
                gi = nc.gpsimd.dma_gather(
                    out_ap=g[:],
                    in_ap=gsrc,
                    idxs_ap=idx_wr[:, k, jh, :],
                    num_idxs=JH,
                    num_idxs_reg=JH,
                    elem_size=4 * C,
                    elem_step=2 * C,
                    transpose=False,
                    single_packet=False,
                    queue_num=q,
                )
                gi.then_inc(sems[q], 16)
                drains[q] += 1
                v_w = nc.vector.wait_ge(sems[q], 16 * drains[q])

                # combine: 4 per-corner contiguous muls (per-partition
                # weights broadcast along c), then 3 adds -> colT [jp, t, c]
                g2 = g2p.tile([128, 4, NTH, C], BF16, tag="g2", name="g2")
                for r in range(4):
                    g_r = bass.AP(tensor=g.tensor, offset=g.offset + r * C,
                                  ap=[g.ap[0], [4 * C, NTH], [2, C // 2],
                                      [1, 2]])
                    wb_r = bass.AP(tensor=wdup.tensor,
                                   offset=(wdup.offset
                                           + ((k * NT + jh * NTH) * 4 + r) * 2),
                                   ap=[wdup.ap[0], [8, NTH], [0, C // 2],
                                       [1, 2]])
                    o_r = bass.AP(tensor=g2.tensor,
                                  offset=g2.offset + r * NTH * C,
                                  ap=[g2.ap[0], [C, NTH], [2, C // 2], [1, 2]])
                    mul = TT_MUL(o_r, g_r, wb_r)
                    if r == 0:
                        add_dep_helper(mul.ins, v_w.ins, sync=False)
                colT = ctp.tile([128, NTH, C], BF16, tag="colT", name="colT")
                TT_ADD(colT, g2[:, 0], g2[:, 1])
                TT_ADD(colT, colT, g2[:, 2])
                TT_ADD(colT, colT, g2[:, 3])

                # PE-transpose to c-major: colC[c, t, jp], j = jh*2048+t*128+jp
                colC = colp.tile([128, NTH, 128], BF16, tag=f"colC{k}",
                                 name=f"colC{k}")
                for h in range(2):
                    tp = ps_t.tile([128, 8, 128], BF16, tag="tp", name="tp")
                    for s in range(8):
                        nc.tensor.transpose(tp[:, s, :], colT[:, 8 * h + s, :],
                                            ident[:])
                    nc.scalar.copy(out=colC[:, 8 * h:8 * (h + 1), :], in_=tp)
                cols.append(colC)

                def emit_gemm(gjh, m, gcols):
                    ps_n = [ps_b.tile([128, 512], F32, tag=f"gemm{n2}",
                                      name=f"gemm{n2}") for n2 in range(4)]
                    for kk in range(K2):
                        ck = gcols[kk][:].rearrange("p t c -> p (t c)")
                        for n2 in range(4):
                            nc.tensor.matmul(
                                ps_n[n2][:],
                                gemm_sb[:, kk, m * 128:(m + 1) * 128],
                                ck[:, n2 * 512:(n2 + 1) * 512],
                                start=(kk == 0), stop=(kk == K2 - 1),
                            )
                    o_sb = outp.tile([128, JH], F32, tag="o", name="o")
                    for n2 in range(4):
                        nc.scalar.activation(
                            out=o_sb[:, n2 * 512:(n2 + 1) * 512], in_=ps_n[n2],
                            func=mybir.ActivationFunctionType.Identity,
                            bias=bias_sb[:, m:m + 1])
                    dst = bass.AP(tensor=y_out,
                                  offset=m * 128 * HW + gjh * JH,
                                  ap=[[HW, 128], [1, JH]])
                    nc.sync.dma_start(out=dst, in_=o_sb)

                # interleave GEMM halves with the next jh's units so the
                # PE burst doesn't stall the transpose chain
                if k == 1 and jh == 1:
                    emit_gemm(0, 1, prev_cols)
                if k == K2 - 1:
                    if jh == 0:
                        emit_gemm(0, 0, cols)
                        prev_cols = cols
                    else:
                        emit_gemm(1, 0, cols)
                        emit_gemm(1, 1, cols)
                    cols = []


def _host_constants():
    if "consts" in _CACHE:
        return _CACHE["consts"]
    t_idx = np.arange(NT)
    p_idx = np.arange(128)
    j = t_idx[None, :] * 128 + p_idx[:, None]          # [128, 32]
    iy = j // W
    ix = j % W
    ky = np.repeat(np.arange(K), K)
    kx = np.tile(np.arange(K), K)
    basey = np.zeros((128, KT), dtype=np.float32)
    basex = np.zeros((128, KT), dtype=np.float32)
    for k in range(K2):
        basey[:, k * NT:(k + 1) * NT] = iy - 1 + ky[k]
        basex[:, k * NT:(k + 1) * NT] = ix - 1 + kx[k]
    _CACHE["consts"] = (basey, basex)
    return _CACHE["consts"]


def kernel(x, offset_w, offset_b, mask_w, mask_b, weights, bias):
    x = np.asarray(x, dtype=np.float32)
    offset_w = np.asarray(offset_w, dtype=np.float32)
    mask_w = np.asarray(mask_w, dtype=np.float32)
    weights = np.asarray(weights, dtype=np.float32)
    bias = np.asarray(bias, dtype=np.float32)
    offset_b = np.asarray(offset_b, dtype=np.float32)
    mask_b = np.asarray(mask_b, dtype=np.float32)
    assert np.all(offset_b == 0) and np.all(mask_b == 0), "zero conv bias assumed"

    if "nc" not in _CACHE:
        _CACHE["nc"] = _build_nc()
    nc = _CACHE["nc"]
    basey, basex = _host_constants()

    # offset/mask conv stationary operand [c, tap, q]: q 0-8 dy, 9-17 dx, 18-26 m
    lhsT_om = np.zeros((C, K2, 32), dtype=np.float32)
    ow = offset_w.reshape(K2, 2, C, K, K)
    for tap in range(K2):
        dy, dx = tap // K, tap % K
        lhsT_om[:, tap, 0:K2] = ow[:, 0, :, dy, dx].T
        lhsT_om[:, tap, K2:2 * K2] = ow[:, 1, :, dy, dx].T
        lhsT_om[:, tap, 2 * K2:3 * K2] = mask_w[:, :, dy, dx].T
    lhsT_om = lhsT_om.astype(ml_dtypes.bfloat16)

    # GEMM stationary operand: lhsT_gemm[k, c, p] = weights[p, c*9 + k]
    wr = weights.reshape(P, C, K2)
    lhsT_gemm = np.ascontiguousarray(wr.transpose(1, 2, 0)).astype(ml_dtypes.bfloat16)

    bias_col = np.ascontiguousarray(bias.reshape(2, 128).T).astype(np.float32)

    in_maps = []
    for n in range(N):
        in_maps.append({
            "x": np.ascontiguousarray(x[n].reshape(C, HW)),
            "lhsT_om": lhsT_om,
            "lhsT_gemm": lhsT_gemm,
            "basey": basey,
            "basex": basex,
            "bias_col": bias_col,
        })

    res = run_bass_kernel_spmd(nc, in_maps, core_ids=list(range(N)),
                               trace=bool(_CACHE.get("trace")),
                               trace_cores=_CACHE.get("trace_cores"))
    _CACHE["last_res"] = res
    out = np.stack([res.results[n]["y"].reshape(P, H, W) for n in range(N)])
    return out.astype(np.float32)


# revision 18
# speedup vs baseline: 1.0680x; 1.0191x over previous
"""Deformable conv (DCNv2) Trainium2 Bass kernel — v2.

Problem (hardcoded): x [8, 128, 64, 64] f32; offset/mask 3x3 convs (pad 1);
bilinear-gather im2col; GEMM with weights [256, 1152]; out [8, 256, 64, 64].

Sharding: data-parallel over batch N=8 across 8 NeuronCores (1 sample/core);
weights/conv params replicated.

Per-core pipeline (sample n):
  1. x -> SBUF; zero-padded bf16 conv input xpad [128, 68*68]; transposed
     padded image xt_pad [68*68(+70), 128] bf16 in DRAM (pad ring = 2 so
     out-of-range bilinear corners read zeros); 4 shifted DRAM->DRAM copies
     build xt4 [4625, 4, 128]: row r = the 4 bilinear corner pixel-rows
     {r, r+1, r+68, r+69} of an anchor, contiguous 1024B.
  2. PE: offset/mask conv as 9 shifted matmuls (27 out rows).
  3. PE-transpose conv output to j-major [j%128 part, (q, t)]; DVE coord
     math: floor, clamp, frac weights; mask*bilinear corner weights packed
     j-major as wpkb [128, k, t, r] bf16 (stay in SBUF — no broadcast);
     int16 anchor index per (tap, j) into xt4, wrapped for the gather.
  4. Per (jh, k) unit (18 total): NON-transpose SWDGE dma_gather (elem
     1024B) -> g [128 j-part, 16, 512] bf16, queues round-robin 0-3 with
     inline trigger (drains overlap freely - no xbar). DVE: one stride-0-
     broadcast mul by wpkb (scattered to corner planes) + 3 adds ->
     colT [j-part, t, c]. PE: 16 transposes (128x128, sub-bank PSUM) + 2
     ACT copies -> colC_k [c, (t, jp)] = im2col block in c-major, columns
     in sequential j order.
  5. PE GEMM accumulates over k; ACT bias epilogue; contiguous f32 store.
"""

import numpy as np
import ml_dtypes

import concourse.bass as bass
import concourse.mybir as mybir
import concourse.tile as tile
from concourse import bacc
from concourse.bass_utils import run_bass_kernel_spmd
from concourse.masks import make_identity
from concourse.tile_rust import add_dep_helper

F32 = mybir.dt.float32
BF16 = mybir.dt.bfloat16
I16 = mybir.dt.int16

N, C, H, W = 8, 128, 64, 64
K, K2, P = 3, 9, 256
HW = H * W                  # 4096
PW = W + 4                  # 68  (pad ring of 2)
ROWS = PW * PW              # 4624
ROWSP = ROWS + 70           # xt_pad rows (so xt4 row r can read r+69)
NT = HW // 128              # 32 j-tiles of 128
KT = K2 * NT                # 288
JH = HW // 2                # 2048 per j-half
NTH = NT // 2               # 16 t-chunks per j-half
MAGIC = 12582912.0          # 1.5 * 2**23: fp32 round-to-int magic

_CACHE = {}


def _build_nc():
    nc = bacc.Bacc("TRN2", target_bir_lowering=False, debug=False,
                   num_devices=N, num_swdge_queues=4)

    x_in = nc.dram_tensor("x", [C, HW], F32, kind="ExternalInput")
    lhsT_om = nc.dram_tensor("lhsT_om", [C, K2, 32], BF16, kind="ExternalInput")
    lhsT_gemm = nc.dram_tensor("lhsT_gemm", [C, K2, P], BF16, kind="ExternalInput")
    basey = nc.dram_tensor("basey", [128, KT], F32, kind="ExternalInput")
    basex = nc.dram_tensor("basex", [128, KT], F32, kind="ExternalInput")
    bias_col = nc.dram_tensor("bias_col", [128, 2], F32, kind="ExternalInput")
    y_out = nc.dram_tensor("y", [P, HW], F32, kind="ExternalOutput")

    with tile.TileContext(nc) as tc:
        with tc.tile_pool(name="dram", bufs=1, space="DRAM") as dram:
            xt4q = dram.tile([ROWS, 4 * C], BF16)
            idxw_dram = dram.tile([16, K2 * 2 * 128], I16)
            _emit(tc, nc, x_in, lhsT_om, lhsT_gemm, basey, basex,
                  bias_col, y_out, xt4q, idxw_dram)
    nc.compile()
    return nc


def _emit(tc, nc, x_in, lhsT_om, lhsT_gemm, basey, basex, bias_col,
          y_out, xt4q, idxw_dram):
    TS = nc.vector.tensor_scalar
    TT_ADD = nc.vector.tensor_add
    TT_SUB = nc.vector.tensor_sub
    TT_MUL = nc.vector.tensor_mul
    Alu = mybir.AluOpType

    with tc.tile_pool(name="singles", bufs=1) as singles:
        # ---- persistent tiles ----
        om_sb = singles.tile([C, K2, 32], BF16, tag="om", name="om")
        gemm_sb = singles.tile([C, K2, P], BF16, tag="gemm_w", name="gemm_w")
        bias_sb = singles.tile([128, 2], F32, tag="bias", name="bias")
        idx_wr = singles.tile([128, K2, 2, 128], I16, tag="idx_wr", name="idx_wr")
        wpkb = singles.tile([128, K2, NT, 4], BF16, tag="wpkb", name="wpkb")
        wdup = singles.tile([128, K2, NT, 4, 2], BF16, tag="wdup", name="wdup")
        ident = singles.tile([128, 128], BF16, tag="ident", name="ident")
        identf = singles.tile([32, 32], F32, tag="identf", name="identf")
        identf128 = singles.tile([128, 128], F32, tag="identf128",
                                 name="identf128")

        nc.sync.dma_start(out=om_sb, in_=lhsT_om[:])
        nc.sync.dma_start(out=gemm_sb, in_=lhsT_gemm[:])
        nc.sync.dma_start(out=bias_sb, in_=bias_col[:])
        make_identity(nc, ident)
        make_identity(nc, identf)
        make_identity(nc, identf128)

        with tc.tile_pool(name="stage1", bufs=1) as st1, \
             tc.tile_pool(name="coord", bufs=1) as coord, \
             tc.tile_pool(name="ps_a", bufs=2, space="PSUM") as ps_a, \
             tc.tile_pool(name="trbuf", bufs=4) as trbuf:

            # ---- stage 1: load x, build xpad (SBUF) and xt_pad (DRAM) ----
            xpad = st1.tile([C, ROWS + 1], BF16, tag="xpad", name="xpad")
            x_sb = st1.tile([C, HW], F32, tag="x", name="x")
            nc.sync.dma_start(out=x_sb, in_=x_in[:])

            nc.vector.memset(xpad, 0.0)
            xpad_int = bass.AP(tensor=xpad.tensor,
                               offset=xpad.offset + 2 * PW + 2,
                               ap=[xpad.ap[0], [PW, H], [1, W]])
            nc.scalar.copy(out=xpad_int,
                           in_=x_sb[:].rearrange("c (h w) -> c h w", h=H))

            # build xt4q[a] = the 4 corner pixel-rows {a, a+1, a+68,
            # a+69} of anchor a, 1024B contiguous, directly from xpad:
            # per padded row-pair u (34), four column-shifted PE
            # transposes put the full quad in each SBUF partition, so the
            # DRAM write uses 1024B descriptors (68/chunk). Pad zeros
            # come from xpad itself - no zero-fill pass needed.
            for u in range(67):
                qp = ps_a.tile([PW, 4, C], BF16, tag="quad", name="quad")
                for r, dlt in enumerate((0, 1, PW, PW + 1)):
                    base = u * PW + dlt
                    nc.tensor.transpose(qp[:, r, :],
                                        xpad[:, base:base + PW], ident[:])
                qs = trbuf.tile([PW, 4, C], BF16, tag="quad_sb",
                                name="quad_sb")
                nc.scalar.copy(out=qs, in_=qp)
                dst = bass.AP(tensor=xt4q.tensor,
                              offset=xt4q.offset + (u * PW) * 4 * C,
                              ap=[[4 * C, PW], [1, 4 * C]])
                src = bass.AP(tensor=qs.tensor, offset=qs.offset,
                              ap=[[qs.ap[0][0], PW], [1, 4 * C]])
                eng = nc.sync if u % 2 == 0 else nc.scalar
                eng.dma_start(out=dst, in_=src)

            # ---- stage 2: offset/mask conv (27 out rows), 512-col chunks ----
            co_sb = st1.tile([32, HW], F32, tag="co", name="co")
            for nt8 in range(8):
                co_ps = ps_a.tile([32, 512], F32, tag="conv", name="conv")
                for tap in range(K2):
                    dy, dx = tap // K, tap % K
                    rhs = bass.AP(
                        tensor=xpad.tensor,
                        offset=(xpad.offset + (1 + dy) * PW + (1 + dx)
                                + (nt8 * 8) * PW),
                        ap=[xpad.ap[0], [PW, 8], [1, W]],
                    )
                    nc.tensor.matmul(co_ps[:], om_sb[:, tap, :], rhs,
                                     start=(tap == 0), stop=(tap == K2 - 1))
                nc.scalar.copy(out=co_sb[:, nt8 * 512:(nt8 + 1) * 512],
                               in_=co_ps)

            # ---- stage 3: transpose conv out to j-major; coordinate math ----
            trj = coord.tile([128, 32, NT], F32, tag="trj", name="trj")   # [jp, q, t]
            for t in range(NT):
                tp = ps_a.tile([128, 32], F32, tag="trjp", name="trjp")
                nc.tensor.transpose(tp[:], co_sb[:, t * 128:(t + 1) * 128],
                                    identf[:])
                nc.vector.tensor_copy(trj[:, :, t], tp)

            dy_all = trj[:, 0:K2, :]
            dx_all = trj[:, K2:2 * K2, :]
            m_all = trj[:, 2 * K2:3 * K2, :]

            by = coord.tile([128, KT], F32, tag="by", name="by")
            bx = coord.tile([128, KT], F32, tag="bx", name="bx")
            nc.sync.dma_start(out=by, in_=basey[:])
            nc.sync.dma_start(out=bx, in_=basex[:])

            def f32t(tag):
                return coord.tile([128, KT], F32, tag=tag, name=tag)

            py = f32t("py"); TT_ADD(py, dy_all, by)
            px = f32t("px"); TT_ADD(px, dx_all, bx)
            ty = f32t("ty"); TS(out=ty, in0=py, scalar1=-0.5, scalar2=MAGIC,
                                op0=Alu.add, op1=Alu.add)
            y0 = f32t("y0"); TS(out=y0, in0=ty, scalar1=MAGIC, scalar2=None,
                                op0=Alu.subtract)
            tx = f32t("tx"); TS(out=tx, in0=px, scalar1=-0.5, scalar2=MAGIC,
                                op0=Alu.add, op1=Alu.add)
            x0 = f32t("x0"); TS(out=x0, in0=tx, scalar1=MAGIC, scalar2=None,
                                op0=Alu.subtract)
            ly = f32t("ly"); TT_SUB(ly, py, y0)
            lx = f32t("lx"); TT_SUB(lx, px, x0)
            y0c = f32t("y0c"); TS(out=y0c, in0=y0, scalar1=-2.0, scalar2=64.0,
                                  op0=Alu.max, op1=Alu.min)
            x0c = f32t("x0c"); TS(out=x0c, in0=x0, scalar1=-2.0, scalar2=64.0,
                                  op0=Alu.max, op1=Alu.min)

            # idx = (y0c+2)*68 + (x0c+2)
            ia = f32t("ia"); TS(out=ia, in0=y0c, scalar1=float(PW),
                                scalar2=float(2 * PW + 2),
                                op0=Alu.mult, op1=Alu.add)
            idx0f = f32t("idx0f"); TT_ADD(idx0f, ia, x0c)

            # mask * bilinear corner weights (mask = 2*sigmoid(conv)),
            # packed j-major into wpack[jp, k, t, r]; corner order matches
            # xt4: r0=(y0,x0) r1=(y0,x0+1) r2=(y0+1,x0) r3=(y0+1,x0+1)
            sig = f32t("sig")
            nc.scalar.activation(out=sig, in_=m_all,
                                 func=mybir.ActivationFunctionType.Sigmoid)
            m2 = f32t("m2"); TS(out=m2, in0=sig, scalar1=2.0, scalar2=None,
                                op0=Alu.mult)
            mly = f32t("mly"); TT_MUL(mly, m2, ly)
            muy = f32t("muy"); TT_SUB(muy, m2, mly)

            wpack = coord.tile([128, K2, NT, 4], F32, tag="wpack", name="wpack")

            def wslice(r):
                return bass.AP(tensor=wpack.tensor,
                               offset=wpack.offset + r,
                               ap=[wpack.ap[0], [NT * 4, K2], [4, NT]])

            def v3(t):  # [128, KT] -> [128, K2, NT] view
                return t[:].rearrange("p (k t) -> p k t", k=K2)

            TT_MUL(wslice(3), v3(mly), v3(lx))
            TT_SUB(wslice(2), v3(mly), wslice(3))
            TT_MUL(wslice(1), v3(muy), v3(lx))
            TT_SUB(wslice(0), v3(muy), wslice(1))
            nc.vector.tensor_copy(wpkb, wpack)
            wde = bass.AP(tensor=wdup.tensor, offset=wdup.offset,
                          ap=[wdup.ap[0], [2, K2 * NT * 4], [1, 2]])
            wps = bass.AP(tensor=wpkb.tensor, offset=wpkb.offset,
                          ap=[wpkb.ap[0], [1, K2 * NT * 4], [0, 2]])
            nc.vector.tensor_copy(wde, wps)

            # wrap indices for the non-transpose gather: descriptor number
            # i = t*128 + p (j = jh*2048 + i) read from wrapped [i%16, i//16]
            # = [p%16, t*8 + p//16] with p = 16a + b. The 8-way a-interleave
            # crosses partition groups, so route through PE transposes:
            # idx0f k-chunks [128, 32] -> [32 t, 128 jp] PSUM, ACT-scatter
            # the jp axis to (b*8 + a) order, cast i16, then 18 clean 3-D
            # SBUF->DRAM writes into wrapped layout + 1 broadcast back.
            idxts = coord.tile([32, K2, 128], F32, tag="idxts", name="idxts")
            for k in range(K2):
                tp2 = ps_a.tile([32, 128], F32, tag="idxtp", name="idxtp")
                nc.tensor.transpose(tp2[:], idx0f[:, k * NT:(k + 1) * NT],
                                    identf128[:])
                # scatter jp = 16a+b -> free pos b*8 + a
                dsc = bass.AP(tensor=idxts.tensor,
                              offset=idxts.offset + k * 128,
                              ap=[idxts.ap[0], [1, 8], [8, 16]])
                nc.scalar.copy(out=dsc, in_=tp2[:].rearrange("t (a b) -> t a b",
                                                             a=8))
            idxts_i = coord.tile([32, K2, 128], I16, tag="idxts_i",
                                 name="idxts_i")
            nc.vector.tensor_copy(idxts_i, idxts)
            for k in range(K2):
                for jh in range(2):
                    sbt = idxts_i[jh * NTH:(jh + 1) * NTH, :, :]
                    src = bass.AP(tensor=sbt.tensor,
                                  offset=sbt.offset + k * 128,
                                  ap=[sbt.ap[0], [8, 16], [1, 8]])
                    dst = bass.AP(tensor=idxw_dram.tensor,
                                  offset=(idxw_dram.offset + k * 256
                                          + jh * 128),
                                  ap=[[8, NTH], [K2 * 256, 16], [1, 8]])
                    nc.sync.dma_start(out=dst, in_=src)
            FW = K2 * 2 * 128
            bsrc = bass.AP(tensor=idxw_dram.tensor, offset=idxw_dram.offset,
                           ap=[[0, 8], [FW, 16], [1, FW]])
            idst = bass.AP(tensor=idx_wr.tensor, offset=idx_wr.offset,
                           ap=[[idx_wr.ap[0][0], 128], [1, FW]])
            nc.sync.dma_start(out=idst, in_=bsrc)

        # ---- stages 4+5: per (jh, k): gather -> combine -> transpose; GEMM
        gsrc = bass.AP(tensor=xt4q.tensor, offset=xt4q.offset,
                       ap=[[4 * C, ROWS], [1, 4 * C]])
        sems = [nc.alloc_semaphore(f"swdge_q{q}") for q in range(4)]
        drains = [0, 0, 0, 0]

        with tc.tile_pool(name="gw", bufs=4) as gw, \
             tc.tile_pool(name="g2p", bufs=1) as g2p, \
             tc.tile_pool(name="ctp", bufs=2) as ctp, \
             tc.tile_pool(name="colp", bufs=2) as colp, \
             tc.tile_pool(name="outp", bufs=2) as outp, \
             tc.tile_pool(name="ps_t", bufs=2, space="PSUM") as ps_t, \
             tc.tile_pool(name="ps_b", bufs=1, space="PSUM") as ps_b:

            cols = []
            for u, (jh, k) in enumerate(
                    [(jh, k) for jh in range(2) for k in range(K2)]):
                q = u % 4
                g = gw.tile([128, NTH, 4 * C], BF16, tag="g", name="g")
                gi = nc.gpsimd.dma_gather(
                    out_ap=g[:],
                    in_ap=gsrc,
                    idxs_ap=idx_wr[:, k, jh, :],
                    num_idxs=JH,
                    num_idxs_reg=JH,
                    elem_size=4 * C,
                    elem_step=4 * C,
                    transpose=False,
                    single_packet=False,
                    queue_num=q,
                )
                gi.then_inc(sems[q], 16)
                drains[q] += 1
                v_w = nc.vector.wait_ge(sems[q], 16 * drains[q])

                # combine: 4 per-corner contiguous muls (per-partition
                # weights broadcast along c), then 3 adds -> colT [jp, t, c]
                g2 = g2p.tile([128, 4, NTH, C], BF16, tag="g2", name="g2")
                for r in range(4):
                    g_r = bass.AP(tensor=g.tensor, offset=g.offset + r * C,
                                  ap=[g.ap[0], [4 * C, NTH], [2, C // 2],
                                      [1, 2]])
                    wb_r = bass.AP(tensor=wdup.tensor,
                                   offset=(wdup.offset
                                           + ((k * NT + jh * NTH) * 4 + r) * 2),
                                   ap=[wdup.ap[0], [8, NTH], [0, C // 2],
                                       [1, 2]])
                    o_r = bass.AP(tensor=g2.tensor,
                                  offset=g2.offset + r * NTH * C,
                                  ap=[g2.ap[0], [C, NTH], [2, C // 2], [1, 2]])
                    mul = TT_MUL(o_r, g_r, wb_r)
                    if r == 0:
                        add_dep_helper(mul.ins, v_w.ins, sync=False)
                colT = ctp.tile([128, NTH, C], BF16, tag="colT", name="colT")
                TT_ADD(colT, g2[:, 0], g2[:, 1])
                TT_ADD(colT, colT, g2[:, 2])
                TT_ADD(colT, colT, g2[:, 3])

                # PE-transpose to c-major: colC[c, t, jp], j = jh*2048+t*128+jp
                colC = colp.tile([128, NTH, 128], BF16, tag=f"colC{k}",
                                 name=f"colC{k}")
                for h in range(2):
                    tp = ps_t.tile([128, 8, 128], BF16, tag="tp", name="tp")
                    for s in range(8):
                        nc.tensor.transpose(tp[:, s, :], colT[:, 8 * h + s, :],
                                            ident[:])
                    nc.scalar.copy(out=colC[:, 8 * h:8 * (h + 1), :], in_=tp)
                cols.append(colC)

                def emit_gemm(gjh, m, gcols):
                    ps_n = [ps_b.tile([128, 512], F32, tag=f"gemm{n2}",
                                      name=f"gemm{n2}") for n2 in range(4)]
                    for kk in range(K2):
                        ck = gcols[kk][:].rearrange("p t c -> p (t c)")
                        for n2 in range(4):
                            nc.tensor.matmul(
                                ps_n[n2][:],
                                gemm_sb[:, kk, m * 128:(m + 1) * 128],
                                ck[:, n2 * 512:(n2 + 1) * 512],
                                start=(kk == 0), stop=(kk == K2 - 1),
                            )
                    o_sb = outp.tile([128, JH], F32, tag="o", name="o")
                    for n2 in range(4):
                        nc.scalar.activation(
                            out=o_sb[:, n2 * 512:(n2 + 1) * 512], in_=ps_n[n2],
                            func=mybir.ActivationFunctionType.Identity,
                            bias=bias_sb[:, m:m + 1])
                    dst = bass.AP(tensor=y_out,
                                  offset=m * 128 * HW + gjh * JH,
                                  ap=[[HW, 128], [1, JH]])
                    nc.sync.dma_start(out=dst, in_=o_sb)

                # interleave GEMM halves with the next jh's units so the
                # PE burst doesn't stall the transpose chain
                if k == 1 and jh == 1:
                    emit_gemm(0, 1, prev_cols)
                if k == K2 - 1:
                    if jh == 0:
                        emit_gemm(0, 0, cols)
                        prev_cols = cols
                    else:
                        emit_gemm(1, 0, cols)
                        emit_gemm(1, 1, cols)
                    cols = []


def _host_constants():
    if "consts" in _CACHE:
        return _CACHE["consts"]
    t_idx = np.arange(NT)
    p_idx = np.arange(128)
    j = t_idx[None, :] * 128 + p_idx[:, None]          # [128, 32]
    iy = j // W
    ix = j % W
    ky = np.repeat(np.arange(K), K)
    kx = np.tile(np.arange(K), K)
    basey = np.zeros((128, KT), dtype=np.float32)
    basex = np.zeros((128, KT), dtype=np.float32)
    for k in range(K2):
        basey[:, k * NT:(k + 1) * NT] = iy - 1 + ky[k]
        basex[:, k * NT:(k + 1) * NT] = ix - 1 + kx[k]
    _CACHE["consts"] = (basey, basex)
    return _CACHE["consts"]


def kernel(x, offset_w, offset_b, mask_w, mask_b, weights, bias):
    x = np.asarray(x, dtype=np.float32)
    offset_w = np.asarray(offset_w, dtype=np.float32)
    mask_w = np.asarray(mask_w, dtype=np.float32)
    weights = np.asarray(weights, dtype=np.float32)
    bias = np.asarray(bias, dtype=np.float32)
    offset_b = np.asarray(offset_b, dtype=np.float32)
    mask_b = np.asarray(mask_b, dtype=np.float32)
    assert np.all(offset_b == 0) and np.all(mask_b == 0), "zero conv bias assumed"

    if "nc" not in _CACHE:
        _CACHE["nc"] = _build_nc()
    nc = _CACHE["nc"]
    basey, basex = _host_constants()

    # offset/mask conv stationary operand [c, tap, q]: q 0-8 dy, 9-17 dx, 18-26 m
    lhsT_om = np.zeros((C, K2, 32), dtype=np.float32)
    ow = offset_w.reshape(K2, 2, C, K, K)
    for tap in range(K2):
        dy, dx = tap // K, tap % K
        lhsT_om[:, tap, 0:K2] = ow[:, 0, :, dy, dx].T
        lhsT_om[:, tap, K2:2 * K2] = ow[:, 1, :, dy, dx].T
        lhsT_om[:, tap, 2 * K2:3 * K2] = mask_w[:, :, dy, dx].T
    lhsT_om = lhsT_om.astype(ml_dtypes.bfloat16)

    # GEMM stationary operand: lhsT_gemm[k, c, p] = weights[p, c*9 + k]
    wr = weights.reshape(P, C, K2)
    lhsT_gemm = np.ascontiguousarray(wr.transpose(1, 2, 0)).astype(ml_dtypes.bfloat16)

    bias_col = np.ascontiguousarray(bias.reshape(2, 128).T).astype(np.float32)

    in_maps = []
    for n in range(N):
        in_maps.append({
            "x": np.ascontiguousarray(x[n].reshape(C, HW)),
            "lhsT_om": lhsT_om,
            "lhsT_gemm": lhsT_gemm,
            "basey": basey,
            "basex": basex,
            "bias_col": bias_col,
        })

    res = run_bass_kernel_spmd(nc, in_maps, core_ids=list(range(N)),
                               trace=bool(_CACHE.get("trace")),
                               trace_cores=_CACHE.get("trace_cores"))
    _CACHE["last_res"] = res
    out = np.stack([res.results[n]["y"].reshape(P, H, W) for n in range(N)])
    return out.astype(np.float32)


# revision 21
# speedup vs baseline: 1.1512x; 1.0779x over previous
"""Deformable conv (DCNv2) Trainium2 Bass kernel — v2.

Problem (hardcoded): x [8, 128, 64, 64] f32; offset/mask 3x3 convs (pad 1);
bilinear-gather im2col; GEMM with weights [256, 1152]; out [8, 256, 64, 64].

Sharding: data-parallel over batch N=8 across 8 NeuronCores (1 sample/core);
weights/conv params replicated.

Per-core pipeline (sample n):
  1. x -> SBUF; zero-padded bf16 conv input xpad [128, 68*68]; transposed
     padded image xt_pad [68*68(+70), 128] bf16 in DRAM (pad ring = 2 so
     out-of-range bilinear corners read zeros); 4 shifted DRAM->DRAM copies
     build xt4 [4625, 4, 128]: row r = the 4 bilinear corner pixel-rows
     {r, r+1, r+68, r+69} of an anchor, contiguous 1024B.
  2. PE: offset/mask conv as 9 shifted matmuls (27 out rows).
  3. PE-transpose conv output to j-major [j%128 part, (q, t)]; DVE coord
     math: floor, clamp, frac weights; mask*bilinear corner weights packed
     j-major as wpkb [128, k, t, r] bf16 (stay in SBUF — no broadcast);
     int16 anchor index per (tap, j) into xt4, wrapped for the gather.
  4. Per (jh, k) unit (18 total): NON-transpose SWDGE dma_gather (elem
     1024B) -> g [128 j-part, 16, 512] bf16, queues round-robin 0-3 with
     inline trigger (drains overlap freely - no xbar). DVE: one stride-0-
     broadcast mul by wpkb (scattered to corner planes) + 3 adds ->
     colT [j-part, t, c]. PE: 16 transposes (128x128, sub-bank PSUM) + 2
     ACT copies -> colC_k [c, (t, jp)] = im2col block in c-major, columns
     in sequential j order.
  5. PE GEMM accumulates over k; ACT bias epilogue; contiguous f32 store.
"""

import numpy as np
import ml_dtypes

import concourse.bass as bass
import concourse.mybir as mybir
import concourse.tile as tile
from concourse import bacc
from concourse.bass_utils import run_bass_kernel_spmd
from concourse.masks import make_identity
from concourse.tile_rust import add_dep_helper

F32 = mybir.dt.float32
BF16 = mybir.dt.bfloat16
I16 = mybir.dt.int16

N, C, H, W = 8, 128, 64, 64
K, K2, P = 3, 9, 256
HW = H * W                  # 4096
PW = W + 4                  # 68  (pad ring of 2)
ROWS = PW * PW              # 4624
ROWSP = ROWS + 70           # xt_pad rows (so xt4 row r can read r+69)
NT = HW // 128              # 32 j-tiles of 128
KT = K2 * NT                # 288
JH = HW // 2                # 2048 per j-half
NTH = NT // 2               # 16 t-chunks per j-half
MAGIC = 12582912.0          # 1.5 * 2**23: fp32 round-to-int magic

_CACHE = {}


def _build_nc():
    nc = bacc.Bacc("TRN2", target_bir_lowering=False, debug=False,
                   num_devices=N, num_swdge_queues=4)

    x_in = nc.dram_tensor("x", [C, HW], F32, kind="ExternalInput")
    lhsT_om = nc.dram_tensor("lhsT_om", [C, K2, 32], BF16, kind="ExternalInput")
    lhsT_gemm = nc.dram_tensor("lhsT_gemm", [C, K2, P], BF16, kind="ExternalInput")
    basey = nc.dram_tensor("basey", [128, KT], F32, kind="ExternalInput")
    basex = nc.dram_tensor("basex", [128, KT], F32, kind="ExternalInput")
    bias_col = nc.dram_tensor("bias_col", [128, 2], F32, kind="ExternalInput")
    y_out = nc.dram_tensor("y", [P, HW], F32, kind="ExternalOutput")

    with tile.TileContext(nc) as tc:
        with tc.tile_pool(name="dram", bufs=1, space="DRAM") as dram:
            xt4q = dram.tile([4608, 4 * C], BF16)
            idxw_dram = dram.tile([16, K2 * 2 * 128], I16)
            _emit(tc, nc, x_in, lhsT_om, lhsT_gemm, basey, basex,
                  bias_col, y_out, xt4q, idxw_dram)
    nc.compile()
    return nc


def _emit(tc, nc, x_in, lhsT_om, lhsT_gemm, basey, basex, bias_col,
          y_out, xt4q, idxw_dram):
    TS = nc.vector.tensor_scalar
    TT_ADD = nc.vector.tensor_add
    TT_SUB = nc.vector.tensor_sub
    TT_MUL = nc.vector.tensor_mul
    Alu = mybir.AluOpType

    with tc.tile_pool(name="singles", bufs=1) as singles:
        # ---- persistent tiles ----
        om_sb = singles.tile([C, K2, 32], BF16, tag="om", name="om")
        gemm_sb = singles.tile([C, K2, P], BF16, tag="gemm_w", name="gemm_w")
        bias_sb = singles.tile([128, 2], F32, tag="bias", name="bias")
        idx_wr = singles.tile([128, K2, 2, 128], I16, tag="idx_wr", name="idx_wr")
        wpkb = singles.tile([128, K2, NT, 4], BF16, tag="wpkb", name="wpkb")
        wdup = singles.tile([128, K2, NT, 4, 2], BF16, tag="wdup", name="wdup")
        ident = singles.tile([128, 128], BF16, tag="ident", name="ident")
        identf = singles.tile([32, 32], F32, tag="identf", name="identf")
        identf128 = singles.tile([128, 128], F32, tag="identf128",
                                 name="identf128")

        idx0 = singles.tile([128, 8], I16, tag="idx0", name="idx0")
        gdummy = singles.tile([128, 4, 128], BF16, tag="gdummy",
                              name="gdummy")
        nc.vector.memset(idx0, 0)
        dsrc = bass.AP(tensor=xt4q.tensor, offset=xt4q.offset,
                       ap=[[128, 32], [1, 128]])
        for q in range(4):
            nc.gpsimd.dma_gather(
                out_ap=gdummy[:, q:q + 1, :], in_ap=dsrc, idxs_ap=idx0[:],
                num_idxs=128, num_idxs_reg=128, elem_size=128,
                elem_step=128, transpose=False, single_packet=False,
                queue_num=q)

        nc.sync.dma_start(out=om_sb, in_=lhsT_om[:])
        nc.sync.dma_start(out=gemm_sb, in_=lhsT_gemm[:])
        nc.sync.dma_start(out=bias_sb, in_=bias_col[:])
        make_identity(nc, ident)
        make_identity(nc, identf)
        make_identity(nc, identf128)

        with tc.tile_pool(name="stage1", bufs=1) as st1, \
             tc.tile_pool(name="coord", bufs=1) as coord, \
             tc.tile_pool(name="ps_a", bufs=2, space="PSUM") as ps_a, \
             tc.tile_pool(name="trbuf", bufs=4) as trbuf:

            # ---- stage 1: load x, build xpad (SBUF) and xt_pad (DRAM) ----
            xpad = st1.tile([C, 4678], BF16, tag="xpad", name="xpad")
            x_sb = st1.tile([C, HW], F32, tag="x", name="x")
            nc.sync.dma_start(out=x_sb, in_=x_in[:])

            nc.vector.memset(xpad, 0.0)
            xpad_int = bass.AP(tensor=xpad.tensor,
                               offset=xpad.offset + 2 * PW + 2,
                               ap=[xpad.ap[0], [PW, H], [1, W]])
            nc.scalar.copy(out=xpad_int,
                           in_=x_sb[:].rearrange("c (h w) -> c h w", h=H))

            # build xt4q[a] = the 4 corner pixel-rows {a, a+1, a+68,
            # a+69} of anchor a, 1024B contiguous, directly from xpad:
            # per padded row-pair u (34), four column-shifted PE
            # transposes put the full quad in each SBUF partition, so the
            # DRAM write uses 1024B descriptors (68/chunk). Pad zeros
            # come from xpad itself - no zero-fill pass needed.
            for u in range(36):
                s0 = 128 * u
                qp = ps_a.tile([128, 4, C], BF16, tag="quad", name="quad")
                for r, dlt in enumerate((0, 1, PW, PW + 1)):
                    nc.tensor.transpose(qp[:, r, :],
                                        xpad[:, s0 + dlt:s0 + dlt + 128],
                                        ident[:])
                qs = trbuf.tile([128, 4, C], BF16, tag="quad_sb",
                                name="quad_sb")
                nc.scalar.copy(out=qs, in_=qp)
                dst = bass.AP(tensor=xt4q.tensor,
                              offset=xt4q.offset + s0 * 4 * C,
                              ap=[[4 * C, 128], [1, 4 * C]])
                src = bass.AP(tensor=qs.tensor, offset=qs.offset,
                              ap=[[qs.ap[0][0], 128], [1, 4 * C]])
                eng = nc.sync if u % 2 == 0 else nc.scalar
                eng.dma_start(out=dst, in_=src)

            # ---- stage 2: offset/mask conv (27 out rows), 512-col chunks ----
            co_sb = st1.tile([32, HW], F32, tag="co", name="co")
            for nt8 in range(8):
                co_ps = ps_a.tile([32, 512], F32, tag="conv", name="conv")
                for tap in range(K2):
                    dy, dx = tap // K, tap % K
                    rhs = bass.AP(
                        tensor=xpad.tensor,
                        offset=(xpad.offset + (1 + dy) * PW + (1 + dx)
                                + (nt8 * 8) * PW),
                        ap=[xpad.ap[0], [PW, 8], [1, W]],
                    )
                    nc.tensor.matmul(co_ps[:], om_sb[:, tap, :], rhs,
                                     start=(tap == 0), stop=(tap == K2 - 1))
                nc.scalar.copy(out=co_sb[:, nt8 * 512:(nt8 + 1) * 512],
                               in_=co_ps)

            # ---- stage 3: transpose conv out to j-major; coordinate math ----
            trj = coord.tile([128, 32, NT], F32, tag="trj", name="trj")   # [jp, q, t]
            for t in range(NT):
                tp = ps_a.tile([128, 32], F32, tag="trjp", name="trjp")
                nc.tensor.transpose(tp[:], co_sb[:, t * 128:(t + 1) * 128],
                                    identf[:])
                nc.vector.tensor_copy(trj[:, :, t], tp)

            dy_all = trj[:, 0:K2, :]
            dx_all = trj[:, K2:2 * K2, :]
            m_all = trj[:, 2 * K2:3 * K2, :]

            by = coord.tile([128, KT], F32, tag="by", name="by")
            bx = coord.tile([128, KT], F32, tag="bx", name="bx")
            nc.sync.dma_start(out=by, in_=basey[:])
            nc.sync.dma_start(out=bx, in_=basex[:])

            def f32t(tag):
                return coord.tile([128, KT], F32, tag=tag, name=tag)

            py = f32t("py"); TT_ADD(py, dy_all, by)
            px = f32t("px"); TT_ADD(px, dx_all, bx)
            ty = f32t("ty"); TS(out=ty, in0=py, scalar1=-0.5, scalar2=MAGIC,
                                op0=Alu.add, op1=Alu.add)
            y0 = f32t("y0"); TS(out=y0, in0=ty, scalar1=MAGIC, scalar2=None,
                                op0=Alu.subtract)
            tx = f32t("tx"); TS(out=tx, in0=px, scalar1=-0.5, scalar2=MAGIC,
                                op0=Alu.add, op1=Alu.add)
            x0 = f32t("x0"); TS(out=x0, in0=tx, scalar1=MAGIC, scalar2=None,
                                op0=Alu.subtract)
            ly = f32t("ly"); TT_SUB(ly, py, y0)
            lx = f32t("lx"); TT_SUB(lx, px, x0)
            y0c = f32t("y0c"); TS(out=y0c, in0=y0, scalar1=-2.0, scalar2=64.0,
                                  op0=Alu.max, op1=Alu.min)
            x0c = f32t("x0c"); TS(out=x0c, in0=x0, scalar1=-2.0, scalar2=64.0,
                                  op0=Alu.max, op1=Alu.min)

            # idx = (y0c+2)*68 + (x0c+2)
            ia = f32t("ia"); TS(out=ia, in0=y0c, scalar1=float(PW),
                                scalar2=float(2 * PW + 2),
                                op0=Alu.mult, op1=Alu.add)
            idx0f = f32t("idx0f"); TT_ADD(idx0f, ia, x0c)

            # mask * bilinear corner weights (mask = 2*sigmoid(conv)),
            # packed j-major into wpack[jp, k, t, r]; corner order matches
            # xt4: r0=(y0,x0) r1=(y0,x0+1) r2=(y0+1,x0) r3=(y0+1,x0+1)
            sig = f32t("sig")
            nc.scalar.activation(out=sig, in_=m_all,
                                 func=mybir.ActivationFunctionType.Sigmoid)
            m2 = f32t("m2"); TS(out=m2, in0=sig, scalar1=2.0, scalar2=None,
                                op0=Alu.mult)
            mly = f32t("mly"); TT_MUL(mly, m2, ly)
            muy = f32t("muy"); TT_SUB(muy, m2, mly)

            wpack = coord.tile([128, K2, NT, 4], F32, tag="wpack", name="wpack")

            def wslice(r):
                return bass.AP(tensor=wpack.tensor,
                               offset=wpack.offset + r,
                               ap=[wpack.ap[0], [NT * 4, K2], [4, NT]])

            def v3(t):  # [128, KT] -> [128, K2, NT] view
                return t[:].rearrange("p (k t) -> p k t", k=K2)

            TT_MUL(wslice(3), v3(mly), v3(lx))
            TT_SUB(wslice(2), v3(mly), wslice(3))
            TT_MUL(wslice(1), v3(muy), v3(lx))
            TT_SUB(wslice(0), v3(muy), wslice(1))
            nc.vector.tensor_copy(wpkb, wpack)
            wde = bass.AP(tensor=wdup.tensor, offset=wdup.offset,
                          ap=[wdup.ap[0], [2, K2 * NT * 4], [1, 2]])
            wps = bass.AP(tensor=wpkb.tensor, offset=wpkb.offset,
                          ap=[wpkb.ap[0], [1, K2 * NT * 4], [0, 2]])
            nc.vector.tensor_copy(wde, wps)

            # wrap indices for the non-transpose gather: descriptor number
            # i = t*128 + p (j = jh*2048 + i) read from wrapped [i%16, i//16]
            # = [p%16, t*8 + p//16] with p = 16a + b. The 8-way a-interleave
            # crosses partition groups, so route through PE transposes:
            # idx0f k-chunks [128, 32] -> [32 t, 128 jp] PSUM, ACT-scatter
            # the jp axis to (b*8 + a) order, cast i16, then 18 clean 3-D
            # SBUF->DRAM writes into wrapped layout + 1 broadcast back.
            idxts = coord.tile([32, K2, 128], F32, tag="idxts", name="idxts")
            for k in range(K2):
                tp2 = ps_a.tile([32, 128], F32, tag="idxtp", name="idxtp")
                nc.tensor.transpose(tp2[:], idx0f[:, k * NT:(k + 1) * NT],
                                    identf128[:])
                # scatter jp = 16a+b -> free pos b*8 + a
                dsc = bass.AP(tensor=idxts.tensor,
                              offset=idxts.offset + k * 128,
                              ap=[idxts.ap[0], [1, 8], [8, 16]])
                nc.scalar.copy(out=dsc, in_=tp2[:].rearrange("t (a b) -> t a b",
                                                             a=8))
            idxts_i = coord.tile([32, K2, 128], I16, tag="idxts_i",
                                 name="idxts_i")
            nc.vector.tensor_copy(idxts_i, idxts)
            for k in range(K2):
                for jh in range(2):
                    sbt = idxts_i[jh * NTH:(jh + 1) * NTH, :, :]
                    src = bass.AP(tensor=sbt.tensor,
                                  offset=sbt.offset + k * 128,
                                  ap=[sbt.ap[0], [8, 16], [1, 8]])
                    dst = bass.AP(tensor=idxw_dram.tensor,
                                  offset=(idxw_dram.offset + k * 256
                                          + jh * 128),
                                  ap=[[8, NTH], [K2 * 256, 16], [1, 8]])
                    nc.sync.dma_start(out=dst, in_=src)
            FW = K2 * 2 * 128
            bsrc = bass.AP(tensor=idxw_dram.tensor, offset=idxw_dram.offset,
                           ap=[[0, 8], [FW, 16], [1, FW]])
            idst = bass.AP(tensor=idx_wr.tensor, offset=idx_wr.offset,
                           ap=[[idx_wr.ap[0][0], 128], [1, FW]])
            nc.sync.dma_start(out=idst, in_=bsrc)

        # ---- stages 4+5: per (jh, k): gather -> combine -> transpose; GEMM
        gsrc = bass.AP(tensor=xt4q.tensor, offset=xt4q.offset,
                       ap=[[4 * C, 4608], [1, 4 * C]])
        sems = [nc.alloc_semaphore(f"swdge_q{q}") for q in range(4)]
        drains = [0, 0, 0, 0]

        with tc.tile_pool(name="gw", bufs=4) as gw, \
             tc.tile_pool(name="g2p", bufs=1) as g2p, \
             tc.tile_pool(name="ctp", bufs=2) as ctp, \
             tc.tile_pool(name="colp", bufs=2) as colp, \
             tc.tile_pool(name="outp", bufs=2) as outp, \
             tc.tile_pool(name="ps_t", bufs=2, space="PSUM") as ps_t, \
             tc.tile_pool(name="ps_b", bufs=1, space="PSUM") as ps_b:

            cols = []
            for u, (jh, k) in enumerate(
                    [(jh, k) for jh in range(2) for k in range(K2)]):
                q = u % 4
                g = gw.tile([128, NTH, 4 * C], BF16, tag="g", name="g")
                gi = nc.gpsimd.dma_gather(
                    out_ap=g[:],
                    in_ap=gsrc,
                    idxs_ap=idx_wr[:, k, jh, :],
                    num_idxs=JH,
                    num_idxs_reg=JH,
                    elem_size=4 * C,
                    elem_step=4 * C,
                    transpose=False,
                    single_packet=False,
                    queue_num=q,
                )
                gi.then_inc(sems[q], 16)
                drains[q] += 1
                v_w = nc.vector.wait_ge(sems[q], 16 * drains[q])

                # combine: 4 per-corner contiguous muls (per-partition
                # weights broadcast along c), then 3 adds -> colT [jp, t, c]
                g2 = g2p.tile([128, 4, NTH, C], BF16, tag="g2", name="g2")
                for r in range(4):
                    g_r = bass.AP(tensor=g.tensor, offset=g.offset + r * C,
                                  ap=[g.ap[0], [4 * C, NTH], [2, C // 2],
                                      [1, 2]])
                    wb_r = bass.AP(tensor=wdup.tensor,
                                   offset=(wdup.offset
                                           + ((k * NT + jh * NTH) * 4 + r) * 2),
                                   ap=[wdup.ap[0], [8, NTH], [0, C // 2],
                                       [1, 2]])
                    o_r = bass.AP(tensor=g2.tensor,
                                  offset=g2.offset + r * NTH * C,
                                  ap=[g2.ap[0], [C, NTH], [2, C // 2], [1, 2]])
                    mul = TT_MUL(o_r, g_r, wb_r)
                    if r == 0:
                        add_dep_helper(mul.ins, v_w.ins, sync=False)
                colT = ctp.tile([128, NTH, C], BF16, tag="colT", name="colT")
                TT_ADD(colT, g2[:, 0], g2[:, 1])
                TT_ADD(colT, colT, g2[:, 2])
                TT_ADD(colT, colT, g2[:, 3])

                # PE-transpose to c-major: colC[c, t, jp], j = jh*2048+t*128+jp
                colC = colp.tile([128, NTH, 128], BF16, tag=f"colC{k}",
                                 name=f"colC{k}")
                for h in range(2):
                    tp = ps_t.tile([128, 8, 128], BF16, tag="tp", name="tp")
                    for s in range(8):
                        nc.tensor.transpose(tp[:, s, :], colT[:, 8 * h + s, :],
                                            ident[:])
                    nc.scalar.copy(out=colC[:, 8 * h:8 * (h + 1), :], in_=tp)
                cols.append(colC)

                def emit_gemm(gjh, m, gcols):
                    ps_n = [ps_b.tile([128, 512], F32, tag=f"gemm{n2}",
                                      name=f"gemm{n2}") for n2 in range(4)]
                    for kk in range(K2):
                        ck = gcols[kk][:].rearrange("p t c -> p (t c)")
                        for n2 in range(4):
                            nc.tensor.matmul(
                                ps_n[n2][:],
                                gemm_sb[:, kk, m * 128:(m + 1) * 128],
                                ck[:, n2 * 512:(n2 + 1) * 512],
                                start=(kk == 0), stop=(kk == K2 - 1),
                            )
                    o_sb = outp.tile([128, JH], F32, tag="o", name="o")
                    for n2 in range(4):
                        nc.scalar.activation(
                            out=o_sb[:, n2 * 512:(n2 + 1) * 512], in_=ps_n[n2],
                            func=mybir.ActivationFunctionType.Identity,
                            bias=bias_sb[:, m:m + 1])
                    dst = bass.AP(tensor=y_out,
                                  offset=m * 128 * HW + gjh * JH,
                                  ap=[[HW, 128], [1, JH]])
                    nc.sync.dma_start(out=dst, in_=o_sb)

                # interleave GEMM halves with the next jh's units so the
                # PE burst doesn't stall the transpose chain
                if k == 1 and jh == 1:
                    emit_gemm(0, 1, prev_cols)
                if k == K2 - 1:
                    if jh == 0:
                        emit_gemm(0, 0, cols)
                        prev_cols = cols
                    else:
                        emit_gemm(1, 0, cols)
                        emit_gemm(1, 1, cols)
                    cols = []


def _host_constants():
    if "consts" in _CACHE:
        return _CACHE["consts"]
    t_idx = np.arange(NT)
    p_idx = np.arange(128)
    j = t_idx[None, :] * 128 + p_idx[:, None]          # [128, 32]
    iy = j // W
    ix = j % W
    ky = np.repeat(np.arange(K), K)
    kx = np.tile(np.arange(K), K)
    basey = np.zeros((128, KT), dtype=np.float32)
    basex = np.zeros((128, KT), dtype=np.float32)
    for k in range(K2):
        basey[:, k * NT:(k + 1) * NT] = iy - 1 + ky[k]
        basex[:, k * NT:(k + 1) * NT] = ix - 1 + kx[k]
    _CACHE["consts"] = (basey, basex)
    return _CACHE["consts"]


def kernel(x, offset_w, offset_b, mask_w, mask_b, weights, bias):
    x = np.asarray(x, dtype=np.float32)
    offset_w = np.asarray(offset_w, dtype=np.float32)
    mask_w = np.asarray(mask_w, dtype=np.float32)
    weights = np.asarray(weights, dtype=np.float32)
    bias = np.asarray(bias, dtype=np.float32)
    offset_b = np.asarray(offset_b, dtype=np.float32)
    mask_b = np.asarray(mask_b, dtype=np.float32)
    assert np.all(offset_b == 0) and np.all(mask_b == 0), "zero conv bias assumed"

    if "nc" not in _CACHE:
        _CACHE["nc"] = _build_nc()
    nc = _CACHE["nc"]
    basey, basex = _host_constants()

    # offset/mask conv stationary operand [c, tap, q]: q 0-8 dy, 9-17 dx, 18-26 m
    lhsT_om = np.zeros((C, K2, 32), dtype=np.float32)
    ow = offset_w.reshape(K2, 2, C, K, K)
    for tap in range(K2):
        dy, dx = tap // K, tap % K
        lhsT_om[:, tap, 0:K2] = ow[:, 0, :, dy, dx].T
        lhsT_om[:, tap, K2:2 * K2] = ow[:, 1, :, dy, dx].T
        lhsT_om[:, tap, 2 * K2:3 * K2] = mask_w[:, :, dy, dx].T
    lhsT_om = lhsT_om.astype(ml_dtypes.bfloat16)

    # GEMM stationary operand: lhsT_gemm[k, c, p] = weights[p, c*9 + k]
    wr = weights.reshape(P, C, K2)
    lhsT_gemm = np.ascontiguousarray(wr.transpose(1, 2, 0)).astype(ml_dtypes.bfloat16)

    bias_col = np.ascontiguousarray(bias.reshape(2, 128).T).astype(np.float32)

    in_maps = []
    for n in range(N):
        in_maps.append({
            "x": np.ascontiguousarray(x[n].reshape(C, HW)),
            "lhsT_om": lhsT_om,
            "lhsT_gemm": lhsT_gemm,
            "basey": basey,
            "basex": basex,
            "bias_col": bias_col,
        })

    res = run_bass_kernel_spmd(nc, in_maps, core_ids=list(range(N)),
                               trace=bool(_CACHE.get("trace")),
                               trace_cores=_CACHE.get("trace_cores"))
    _CACHE["last_res"] = res
    out = np.stack([res.results[n]["y"].reshape(P, H, W) for n in range(N)])
    return out.astype(np.float32)


# revision 23
# speedup vs baseline: 1.2061x; 1.0476x over previous
"""Deformable conv (DCNv2) Trainium2 Bass kernel — v2.

Problem (hardcoded): x [8, 128, 64, 64] f32; offset/mask 3x3 convs (pad 1);
bilinear-gather im2col; GEMM with weights [256, 1152]; out [8, 256, 64, 64].

Sharding: data-parallel over batch N=8 across 8 NeuronCores (1 sample/core);
weights/conv params replicated.

Per-core pipeline (sample n):
  1. x -> SBUF; zero-padded bf16 conv input xpad [128, 68*68]; transposed
     padded image xt_pad [68*68(+70), 128] bf16 in DRAM (pad ring = 2 so
     out-of-range bilinear corners read zeros); 4 shifted DRAM->DRAM copies
     build xt4 [4625, 4, 128]: row r = the 4 bilinear corner pixel-rows
     {r, r+1, r+68, r+69} of an anchor, contiguous 1024B.
  2. PE: offset/mask conv as 9 shifted matmuls (27 out rows).
  3. PE-transpose conv output to j-major [j%128 part, (q, t)]; DVE coord
     math: floor, clamp, frac weights; mask*bilinear corner weights packed
     j-major as wpkb [128, k, t, r] bf16 (stay in SBUF — no broadcast);
     int16 anchor index per (tap, j) into xt4, wrapped for the gather.
  4. Per (jh, k) unit (18 total): NON-transpose SWDGE dma_gather (elem
     1024B) -> g [128 j-part, 16, 512] bf16, queues round-robin 0-3 with
     inline trigger (drains overlap freely - no xbar). DVE: one stride-0-
     broadcast mul by wpkb (scattered to corner planes) + 3 adds ->
     colT [j-part, t, c]. PE: 16 transposes (128x128, sub-bank PSUM) + 2
     ACT copies -> colC_k [c, (t, jp)] = im2col block in c-major, columns
     in sequential j order.
  5. PE GEMM accumulates over k; ACT bias epilogue; contiguous f32 store.
"""

import numpy as np
import ml_dtypes

import concourse.bass as bass
import concourse.mybir as mybir
import concourse.tile as tile
from concourse import bacc
from concourse.bass_utils import run_bass_kernel_spmd
from concourse.masks import make_identity
from concourse.tile_rust import add_dep_helper

F32 = mybir.dt.float32
BF16 = mybir.dt.bfloat16
I16 = mybir.dt.int16

N, C, H, W = 8, 128, 64, 64
K, K2, P = 3, 9, 256
HW = H * W                  # 4096
PW = W + 4                  # 68  (pad ring of 2)
ROWS = PW * PW              # 4624
ROWSP = ROWS + 70           # xt_pad rows (so xt4 row r can read r+69)
NT = HW // 128              # 32 j-tiles of 128
KT = K2 * NT                # 288
JH = HW // 2                # 2048 per j-half
NTH = NT // 2               # 16 t-chunks per j-half
MAGIC = 12582912.0          # 1.5 * 2**23: fp32 round-to-int magic

_CACHE = {}


def _build_nc():
    nc = bacc.Bacc("TRN2", target_bir_lowering=False, debug=False,
                   num_devices=N, num_swdge_queues=4)

    x_in = nc.dram_tensor("x", [C, HW], F32, kind="ExternalInput")
    lhsT_om = nc.dram_tensor("lhsT_om", [C, K2, 32], BF16, kind="ExternalInput")
    lhsT_gemm = nc.dram_tensor("lhsT_gemm", [C, K2, P], BF16, kind="ExternalInput")
    basey = nc.dram_tensor("basey", [128, KT], F32, kind="ExternalInput")
    basex = nc.dram_tensor("basex", [128, KT], F32, kind="ExternalInput")
    bias_col = nc.dram_tensor("bias_col", [128, 2], F32, kind="ExternalInput")
    y_out = nc.dram_tensor("y", [P, HW], F32, kind="ExternalOutput")

    with tile.TileContext(nc) as tc:
        with tc.tile_pool(name="dram", bufs=1, space="DRAM") as dram:
            xt4q = dram.tile([4608, 4 * C], BF16)
            idxw_dram = [dram.tile([16, 2 * 128], I16, name=f"idxw{k}")
                         for k in range(K2)]
            _emit(tc, nc, x_in, lhsT_om, lhsT_gemm, basey, basex,
                  bias_col, y_out, xt4q, idxw_dram)
    nc.compile()
    return nc


def _emit(tc, nc, x_in, lhsT_om, lhsT_gemm, basey, basex, bias_col,
          y_out, xt4q, idxw_dram):
    TS = nc.vector.tensor_scalar
    TT_ADD = nc.vector.tensor_add
    TT_SUB = nc.vector.tensor_sub
    TT_MUL = nc.vector.tensor_mul
    Alu = mybir.AluOpType

    with tc.tile_pool(name="singles", bufs=1) as singles:
        # ---- persistent tiles ----
        om_sb = singles.tile([C, K2, 32], BF16, tag="om", name="om")
        gemm_sb = singles.tile([C, K2, P], BF16, tag="gemm_w", name="gemm_w")
        bias_sb = singles.tile([128, 2], F32, tag="bias", name="bias")
        idx_wr = singles.tile([128, K2, 2, 128], I16, tag="idx_wr", name="idx_wr")
        wpkb = singles.tile([128, K2, NT, 4], BF16, tag="wpkb", name="wpkb")
        wdup = singles.tile([128, K2, NT, 4, 2], BF16, tag="wdup", name="wdup")
        ident = singles.tile([128, 128], BF16, tag="ident", name="ident")
        identf = singles.tile([32, 32], F32, tag="identf", name="identf")
        identf128 = singles.tile([128, 128], F32, tag="identf128",
                                 name="identf128")

        idx0 = singles.tile([128, 8], I16, tag="idx0", name="idx0")
        gdummy = singles.tile([128, 4, 128], BF16, tag="gdummy",
                              name="gdummy")
        nc.vector.memset(idx0, 0)
        dsrc = bass.AP(tensor=xt4q.tensor, offset=xt4q.offset,
                       ap=[[128, 32], [1, 128]])
        for q in range(4):
            nc.gpsimd.dma_gather(
                out_ap=gdummy[:, q:q + 1, :], in_ap=dsrc, idxs_ap=idx0[:],
                num_idxs=128, num_idxs_reg=128, elem_size=128,
                elem_step=128, transpose=False, single_packet=False,
                queue_num=q)

        nc.sync.dma_start(out=om_sb, in_=lhsT_om[:])
        nc.sync.dma_start(out=gemm_sb, in_=lhsT_gemm[:])
        nc.sync.dma_start(out=bias_sb, in_=bias_col[:])
        make_identity(nc, ident)
        make_identity(nc, identf)
        make_identity(nc, identf128)

        with tc.tile_pool(name="stage1", bufs=1) as st1, \
             tc.tile_pool(name="coord", bufs=1) as coord, \
             tc.tile_pool(name="ps_a", bufs=2, space="PSUM") as ps_a, \
             tc.tile_pool(name="trbuf", bufs=4) as trbuf:

            # ---- stage 1: load x, build xpad (SBUF) and xt_pad (DRAM) ----
            xpad = st1.tile([C, 4678], BF16, tag="xpad", name="xpad")
            x_sb = st1.tile([C, HW], F32, tag="x", name="x")
            nc.sync.dma_start(out=x_sb, in_=x_in[:])

            nc.vector.memset(xpad, 0.0)
            xpad_int = bass.AP(tensor=xpad.tensor,
                               offset=xpad.offset + 2 * PW + 2,
                               ap=[xpad.ap[0], [PW, H], [1, W]])
            nc.scalar.copy(out=xpad_int,
                           in_=x_sb[:].rearrange("c (h w) -> c h w", h=H))

            # build xt4q[a] = the 4 corner pixel-rows {a, a+1, a+68,
            # a+69} of anchor a, 1024B contiguous, directly from xpad:
            # per padded row-pair u (34), four column-shifted PE
            # transposes put the full quad in each SBUF partition, so the
            # DRAM write uses 1024B descriptors (68/chunk). Pad zeros
            # come from xpad itself - no zero-fill pass needed.
            for u in range(36):
                s0 = 128 * u
                qp = ps_a.tile([128, 4, C], BF16, tag="quad", name="quad")
                for r, dlt in enumerate((0, 1, PW, PW + 1)):
                    nc.tensor.transpose(qp[:, r, :],
                                        xpad[:, s0 + dlt:s0 + dlt + 128],
                                        ident[:])
                qs = trbuf.tile([128, 4, C], BF16, tag="quad_sb",
                                name="quad_sb")
                nc.scalar.copy(out=qs, in_=qp)
                dst = bass.AP(tensor=xt4q.tensor,
                              offset=xt4q.offset + s0 * 4 * C,
                              ap=[[4 * C, 128], [1, 4 * C]])
                src = bass.AP(tensor=qs.tensor, offset=qs.offset,
                              ap=[[qs.ap[0][0], 128], [1, 4 * C]])
                eng = nc.sync if u % 2 == 0 else nc.scalar
                eng.dma_start(out=dst, in_=src)

            # ---- stage 2: offset/mask conv (27 out rows), 512-col chunks ----
            co_sb = st1.tile([32, HW], F32, tag="co", name="co")
            for nt8 in range(8):
                co_ps = ps_a.tile([32, 512], F32, tag="conv", name="conv")
                for tap in range(K2):
                    dy, dx = tap // K, tap % K
                    rhs = bass.AP(
                        tensor=xpad.tensor,
                        offset=(xpad.offset + (1 + dy) * PW + (1 + dx)
                                + (nt8 * 8) * PW),
                        ap=[xpad.ap[0], [PW, 8], [1, W]],
                    )
                    nc.tensor.matmul(co_ps[:], om_sb[:, tap, :], rhs,
                                     start=(tap == 0), stop=(tap == K2 - 1))
                nc.scalar.copy(out=co_sb[:, nt8 * 512:(nt8 + 1) * 512],
                               in_=co_ps)

            # ---- stage 3: transpose conv out to j-major; coordinate math ----
            trj = coord.tile([128, 32, NT], F32, tag="trj", name="trj")   # [jp, q, t]
            for t in range(NT):
                tp = ps_a.tile([128, 32], F32, tag="trjp", name="trjp")
                nc.tensor.transpose(tp[:], co_sb[:, t * 128:(t + 1) * 128],
                                    identf[:])
                nc.vector.tensor_copy(trj[:, :, t], tp)

            dy_all = trj[:, 0:K2, :]
            dx_all = trj[:, K2:2 * K2, :]
            m_all = trj[:, 2 * K2:3 * K2, :]

            by = coord.tile([128, KT], F32, tag="by", name="by")
            bx = coord.tile([128, KT], F32, tag="bx", name="bx")
            nc.sync.dma_start(out=by, in_=basey[:])
            nc.sync.dma_start(out=bx, in_=basex[:])

            def f32t(tag):
                return coord.tile([128, KT], F32, tag=tag, name=tag)

            py = f32t("py"); TT_ADD(py, dy_all, by)
            px = f32t("px"); TT_ADD(px, dx_all, bx)
            ty = f32t("ty"); TS(out=ty, in0=py, scalar1=-0.5, scalar2=MAGIC,
                                op0=Alu.add, op1=Alu.add)
            y0 = f32t("y0"); TS(out=y0, in0=ty, scalar1=MAGIC, scalar2=None,
                                op0=Alu.subtract)
            tx = f32t("tx"); TS(out=tx, in0=px, scalar1=-0.5, scalar2=MAGIC,
                                op0=Alu.add, op1=Alu.add)
            x0 = f32t("x0"); TS(out=x0, in0=tx, scalar1=MAGIC, scalar2=None,
                                op0=Alu.subtract)
            ly = f32t("ly"); TT_SUB(ly, py, y0)
            lx = f32t("lx"); TT_SUB(lx, px, x0)
            y0c = f32t("y0c"); TS(out=y0c, in0=y0, scalar1=-2.0, scalar2=64.0,
                                  op0=Alu.max, op1=Alu.min)
            x0c = f32t("x0c"); TS(out=x0c, in0=x0, scalar1=-2.0, scalar2=64.0,
                                  op0=Alu.max, op1=Alu.min)

            # idx = (y0c+2)*68 + (x0c+2)
            ia = f32t("ia"); TS(out=ia, in0=y0c, scalar1=float(PW),
                                scalar2=float(2 * PW + 2),
                                op0=Alu.mult, op1=Alu.add)
            idx0f = f32t("idx0f"); TT_ADD(idx0f, ia, x0c)

            # mask * bilinear corner weights (mask = 2*sigmoid(conv)),
            # packed j-major into wpack[jp, k, t, r]; corner order matches
            # xt4: r0=(y0,x0) r1=(y0,x0+1) r2=(y0+1,x0) r3=(y0+1,x0+1)
            sig = f32t("sig")
            nc.scalar.activation(out=sig, in_=m_all,
                                 func=mybir.ActivationFunctionType.Sigmoid)
            m2 = f32t("m2"); TS(out=m2, in0=sig, scalar1=2.0, scalar2=None,
                                op0=Alu.mult)
            mly = f32t("mly"); TT_MUL(mly, m2, ly)
            muy = f32t("muy"); TT_SUB(muy, m2, mly)

            wpack = coord.tile([128, K2, NT, 4], F32, tag="wpack", name="wpack")

            def wslice(r):
                return bass.AP(tensor=wpack.tensor,
                               offset=wpack.offset + r,
                               ap=[wpack.ap[0], [NT * 4, K2], [4, NT]])

            def v3(t):  # [128, KT] -> [128, K2, NT] view
                return t[:].rearrange("p (k t) -> p k t", k=K2)

            TT_MUL(wslice(3), v3(mly), v3(lx))
            TT_SUB(wslice(2), v3(mly), wslice(3))
            TT_MUL(wslice(1), v3(muy), v3(lx))
            TT_SUB(wslice(0), v3(muy), wslice(1))
            nc.vector.tensor_copy(wpkb, wpack)
            wde = bass.AP(tensor=wdup.tensor, offset=wdup.offset,
                          ap=[wdup.ap[0], [2, K2 * NT * 4], [1, 2]])
            wps = bass.AP(tensor=wpkb.tensor, offset=wpkb.offset,
                          ap=[wpkb.ap[0], [1, K2 * NT * 4], [0, 2]])
            nc.vector.tensor_copy(wde, wps)

            # wrap indices for the non-transpose gather: descriptor number
            # i = t*128 + p (j = jh*2048 + i) read from wrapped [i%16, i//16]
            # = [p%16, t*8 + p//16] with p = 16a + b. The 8-way a-interleave
            # crosses partition groups, so route through PE transposes:
            # idx0f k-chunks [128, 32] -> [32 t, 128 jp] PSUM, ACT-scatter
            # the jp axis to (b*8 + a) order, cast i16, then 18 clean 3-D
            # SBUF->DRAM writes into wrapped layout + 1 broadcast back.
            idxts = coord.tile([32, K2, 128], F32, tag="idxts", name="idxts")
            for k in range(K2):
                tp2 = ps_a.tile([32, 128], F32, tag="idxtp", name="idxtp")
                nc.tensor.transpose(tp2[:], idx0f[:, k * NT:(k + 1) * NT],
                                    identf128[:])
                # scatter jp = 16a+b -> free pos b*8 + a
                dsc = bass.AP(tensor=idxts.tensor,
                              offset=idxts.offset + k * 128,
                              ap=[idxts.ap[0], [1, 8], [8, 16]])
                nc.scalar.copy(out=dsc, in_=tp2[:].rearrange("t (a b) -> t a b",
                                                             a=8))
            idxts_i = coord.tile([32, K2, 128], I16, tag="idxts_i",
                                 name="idxts_i")
            nc.vector.tensor_copy(idxts_i, idxts)
            for k in range(K2):
                for jh in range(2):
                    sbt = idxts_i[jh * NTH:(jh + 1) * NTH, :, :]
                    src = bass.AP(tensor=sbt.tensor,
                                  offset=sbt.offset + k * 128,
                                  ap=[sbt.ap[0], [8, 16], [1, 8]])
                    dst = bass.AP(tensor=idxw_dram[k].tensor,
                                  offset=idxw_dram[k].offset + jh * 128,
                                  ap=[[8, NTH], [256, 16], [1, 8]])
                    eng = nc.sync if (k + jh) % 2 == 0 else nc.scalar
                    eng.dma_start(out=dst, in_=src)
            for k in range(K2):
                bsrc = bass.AP(tensor=idxw_dram[k].tensor,
                               offset=idxw_dram[k].offset,
                               ap=[[0, 8], [256, 16], [1, 256]])
                kd = bass.AP(tensor=idx_wr.tensor,
                             offset=idx_wr.offset + k * 256,
                             ap=[[idx_wr.ap[0][0], 128], [1, 256]])
                eng = nc.sync if k % 2 == 0 else nc.scalar
                eng.dma_start(out=kd, in_=bsrc)

        # ---- stages 4+5: per (jh, k): gather -> combine -> transpose; GEMM
        gsrc = bass.AP(tensor=xt4q.tensor, offset=xt4q.offset,
                       ap=[[4 * C, 4608], [1, 4 * C]])
        sems = [nc.alloc_semaphore(f"swdge_q{q}") for q in range(4)]
        drains = [0, 0, 0, 0]

        with tc.tile_pool(name="gw", bufs=4) as gw, \
             tc.tile_pool(name="g2p", bufs=1) as g2p, \
             tc.tile_pool(name="ctp", bufs=2) as ctp, \
             tc.tile_pool(name="colp", bufs=2) as colp, \
             tc.tile_pool(name="outp", bufs=2) as outp, \
             tc.tile_pool(name="ps_t", bufs=2, space="PSUM") as ps_t, \
             tc.tile_pool(name="ps_b", bufs=1, space="PSUM") as ps_b:

            cols = []
            for u, (jh, k) in enumerate(
                    [(jh, k) for jh in range(2) for k in range(K2)]):
                q = u % 4
                g = gw.tile([128, NTH, 4 * C], BF16, tag="g", name="g")
                gi = nc.gpsimd.dma_gather(
                    out_ap=g[:],
                    in_ap=gsrc,
                    idxs_ap=idx_wr[:, k, jh, :],
                    num_idxs=JH,
                    num_idxs_reg=JH,
                    elem_size=4 * C,
                    elem_step=4 * C,
                    transpose=False,
                    single_packet=False,
                    queue_num=q,
                )
                gi.then_inc(sems[q], 16)
                drains[q] += 1
                v_w = nc.vector.wait_ge(sems[q], 16 * drains[q])

                # combine: 4 per-corner contiguous muls (per-partition
                # weights broadcast along c), then 3 adds -> colT [jp, t, c]
                g2 = g2p.tile([128, 4, NTH, C], BF16, tag="g2", name="g2")
                for r in range(4):
                    g_r = bass.AP(tensor=g.tensor, offset=g.offset + r * C,
                                  ap=[g.ap[0], [4 * C, NTH], [2, C // 2],
                                      [1, 2]])
                    wb_r = bass.AP(tensor=wdup.tensor,
                                   offset=(wdup.offset
                                           + ((k * NT + jh * NTH) * 4 + r) * 2),
                                   ap=[wdup.ap[0], [8, NTH], [0, C // 2],
                                       [1, 2]])
                    o_r = bass.AP(tensor=g2.tensor,
                                  offset=g2.offset + r * NTH * C,
                                  ap=[g2.ap[0], [C, NTH], [2, C // 2], [1, 2]])
                    mul = TT_MUL(o_r, g_r, wb_r)
                    if r == 0:
                        add_dep_helper(mul.ins, v_w.ins, sync=False)
                colT = ctp.tile([128, NTH, C], BF16, tag="colT", name="colT")
                TT_ADD(colT, g2[:, 0], g2[:, 1])
                TT_ADD(colT, colT, g2[:, 2])
                TT_ADD(colT, colT, g2[:, 3])

                # PE-transpose to c-major: colC[c, t, jp], j = jh*2048+t*128+jp
                colC = colp.tile([128, NTH, 128], BF16, tag=f"colC{k}",
                                 name=f"colC{k}")
                for h in range(2):
                    tp = ps_t.tile([128, 8, 128], BF16, tag="tp", name="tp")
                    for s in range(8):
                        nc.tensor.transpose(tp[:, s, :], colT[:, 8 * h + s, :],
                                            ident[:])
                    nc.scalar.copy(out=colC[:, 8 * h:8 * (h + 1), :], in_=tp)
                cols.append(colC)

                def emit_gemm(gjh, m, gcols):
                    ps_n = [ps_b.tile([128, 512], F32, tag=f"gemm{n2}",
                                      name=f"gemm{n2}") for n2 in range(4)]
                    for kk in range(K2):
                        ck = gcols[kk][:].rearrange("p t c -> p (t c)")
                        for n2 in range(4):
                            nc.tensor.matmul(
                                ps_n[n2][:],
                                gemm_sb[:, kk, m * 128:(m + 1) * 128],
                                ck[:, n2 * 512:(n2 + 1) * 512],
                                start=(kk == 0), stop=(kk == K2 - 1),
                            )
                    o_sb = outp.tile([128, JH], F32, tag="o", name="o")
                    for n2 in range(4):
                        nc.scalar.activation(
                            out=o_sb[:, n2 * 512:(n2 + 1) * 512], in_=ps_n[n2],
                            func=mybir.ActivationFunctionType.Identity,
                            bias=bias_sb[:, m:m + 1])
                    dst = bass.AP(tensor=y_out,
                                  offset=m * 128 * HW + gjh * JH,
                                  ap=[[HW, 128], [1, JH]])
                    nc.sync.dma_start(out=dst, in_=o_sb)

                # interleave GEMM halves with the next jh's units so the
                # PE burst doesn't stall the transpose chain
                if k == 1 and jh == 1:
                    emit_gemm(0, 1, prev_cols)
                if k == K2 - 1:
                    if jh == 0:
                        emit_gemm(0, 0, cols)
                        prev_cols = cols
                    else:
                        emit_gemm(1, 0, cols)
                        emit_gemm(1, 1, cols)
                    cols = []


def _host_constants():
    if "consts" in _CACHE:
        return _CACHE["consts"]
    t_idx = np.arange(NT)
    p_idx = np.arange(128)
    j = t_idx[None, :] * 128 + p_idx[:, None]          # [128, 32]
    iy = j // W
    ix = j % W
    ky = np.repeat(np.arange(K), K)
    kx = np.tile(np.arange(K), K)
    basey = np.zeros((128, KT), dtype=np.float32)
    basex = np.zeros((128, KT), dtype=np.float32)
    for k in range(K2):
        basey[:, k * NT:(k + 1) * NT] = iy - 1 + ky[k]
        basex[:, k * NT:(k + 1) * NT] = ix - 1 + kx[k]
    _CACHE["consts"] = (basey, basex)
    return _CACHE["consts"]


def kernel(x, offset_w, offset_b, mask_w, mask_b, weights, bias):
    x = np.asarray(x, dtype=np.float32)
    offset_w = np.asarray(offset_w, dtype=np.float32)
    mask_w = np.asarray(mask_w, dtype=np.float32)
    weights = np.asarray(weights, dtype=np.float32)
    bias = np.asarray(bias, dtype=np.float32)
    offset_b = np.asarray(offset_b, dtype=np.float32)
    mask_b = np.asarray(mask_b, dtype=np.float32)
    assert np.all(offset_b == 0) and np.all(mask_b == 0), "zero conv bias assumed"

    if "nc" not in _CACHE:
        _CACHE["nc"] = _build_nc()
    nc = _CACHE["nc"]
    basey, basex = _host_constants()

    # offset/mask conv stationary operand [c, tap, q]: q 0-8 dy, 9-17 dx, 18-26 m
    lhsT_om = np.zeros((C, K2, 32), dtype=np.float32)
    ow = offset_w.reshape(K2, 2, C, K, K)
    for tap in range(K2):
        dy, dx = tap // K, tap % K
        lhsT_om[:, tap, 0:K2] = ow[:, 0, :, dy, dx].T
        lhsT_om[:, tap, K2:2 * K2] = ow[:, 1, :, dy, dx].T
        lhsT_om[:, tap, 2 * K2:3 * K2] = mask_w[:, :, dy, dx].T
    lhsT_om = lhsT_om.astype(ml_dtypes.bfloat16)

    # GEMM stationary operand: lhsT_gemm[k, c, p] = weights[p, c*9 + k]
    wr = weights.reshape(P, C, K2)
    lhsT_gemm = np.ascontiguousarray(wr.transpose(1, 2, 0)).astype(ml_dtypes.bfloat16)

    bias_col = np.ascontiguousarray(bias.reshape(2, 128).T).astype(np.float32)

    in_maps = []
    for n in range(N):
        in_maps.append({
            "x": np.ascontiguousarray(x[n].reshape(C, HW)),
            "lhsT_om": lhsT_om,
            "lhsT_gemm": lhsT_gemm,
            "basey": basey,
            "basex": basex,
            "bias_col": bias_col,
        })

    res = run_bass_kernel_spmd(nc, in_maps, core_ids=list(range(N)),
                               trace=bool(_CACHE.get("trace")),
                               trace_cores=_CACHE.get("trace_cores"))
    _CACHE["last_res"] = res
    out = np.stack([res.results[n]["y"].reshape(P, H, W) for n in range(N)])
    return out.astype(np.float32)
